# revision 24
# baseline (speedup 1.0000x reference)
"""Trainium2 Bass kernel for nn_Attention_11836929868370.

8-core sharding: core c -> batch b = c//2, head group hg = c%2 (4 of 8 heads).
Each core computes its 4 heads' attention and a partial output projection;
the host sums the two partials per batch and adds the output bias.

Per-core pipeline (all matmuls bf16, accumulation fp32 in PSUM):
  B (4-nt batches, pipelined): qkv = xT.T @ WqkvT with host-built "mean
      columns" so the per-head LN mean comes out of the matmul for free;
      DVE evacuates t=(q|k)-mu, ACT evacuates v; batched Newton rsqrt per
      4-nt group (so rope/transposes start early); RoPE via rotate-half with
      pre-permuted weights; q''/k'' transposed to [d, n] via xbar DMA
      transposes on the sync (q) and scalar (k) queues.
  C (software-pipelined blocks of (qb, head-pair)): per kt, S^T = k'' @ q''.T
      for both heads of the pair back-to-back — their lhsT base partitions
      (0/64) map to distinct PE row groups, so the K=64 matmuls run
      concurrently in the array. exp(S) runs on ACT for head hh=0 and as a
      one-instruction Schraudolph bitcast-exp on DVE (int16 out, bf16 bit
      pattern) for hh=1 — splitting the softmax-exp load across two engines.
      PV (stationary [v_h | ones]) for the PREVIOUS block interleaves with
      the current block's S matmuls to keep the PE dense; PV row 64 is the
      softmax denominator. Block tail: reciprocal_approx_fast on the
      denominator row, DMA-broadcast across 64 partitions, one DVE multiply.
  D.  out = oT.T @ WoT partial projection per q-block (ACT evacuation),
      DMA out. Host adds out_b (+ the v-bias contribution).
"""

import sys

if "/opt/trn_rl_repo" not in sys.path:
    sys.path.insert(0, "/opt/trn_rl_repo")

import math
from contextlib import ExitStack

import ml_dtypes
import numpy as np

import concourse.bass as bass
import concourse.mybir as mybir
import concourse.tile as tile
from concourse.bass_utils import run_bass_kernel_spmd

BF16 = mybir.dt.bfloat16
F32 = mybir.dt.float32
I16 = mybir.dt.int16
I32 = mybir.dt.int32

DIM, NH, HD = 512, 8, 64
N = 2048
EPS = 1e-6
THETA = 10000.0
NT = N // 128          # 16 n-tiles
CT = DIM // 128        # 4 c-tiles
NHC = 4                # heads per core
QB = 4                 # q blocks of 512
KT = NT                # key tiles
QBW = N // QB          # 512
NTQ = NT // QB         # 4 n-tiles per q block
RSQRT_MAGIC = float(0x5F3759DF)

# Schraudolph exp on DVE: e ~= bitcast_bf16(int16(s * rs * A + B))
EXP_A = 128.0 / math.log(2.0)
EXP_C = 5.0
EXP_B = 16256.0 - EXP_C
# number of kt tiles per block whose hh=1 exp runs on DVE (rest on ACT)
import os as _os

DVE_EXP_KT = int(_os.environ.get("K_DVE_EXP_KT", "16"))
USE_BOUNCE = int(_os.environ.get("K_USE_BOUNCE", "1"))


# ---------------------------------------------------------------------------
# sync-wait legalization: this walrus build rejects >1 sync wait per
# instruction; excess waits are hoisted onto NoOps placed immediately before
# the instruction on the same engine, which preserves ordering exactly.
# ---------------------------------------------------------------------------

def legalize_sync_waits(nc, max_waits=1):
    n = 0
    for fn in nc.m.functions:
        for bb in fn.blocks:
            new_insts = []
            for inst in bb.instructions:
                si = inst.sync_info
                if si is not None and si.on_wait and len(si.on_wait) > max_waits:
                    movable = [w for w in si.on_wait if w.wait_reg is None]
                    pinned = [w for w in si.on_wait if w.wait_reg is not None]
                    budget = max(max_waits - len(pinned), 0)
                    cut = len(movable) - budget
                    keep, excess = movable[cut:], movable[:cut]
                    for i in range(0, len(excess), max_waits):
                        nop = mybir.InstNoOp(
                            name=f"I-waitsplit-{n}",
                            engine=inst.engine,
                            text_hint="waitsplit",
                            sync_info=mybir.SyncInfo(
                                on_wait=excess[i : i + max_waits], on_update=[]
                            ),
                        )
                        n += 1
                        new_insts.append(nop)
                    si.on_wait = keep + pinned
                new_insts.append(inst)
            bb.instructions[:] = new_insts
    return n


# ---------------------------------------------------------------------------
# device program
# ---------------------------------------------------------------------------

def build_program(with_qkv_bias=False, with_ln_bias=False):
    nc = bass.Bass("TRN2", target_bir_lowering=False, debug=False, num_devices=8)

    # [128, CT, 2048]: x transposed (c on partitions) and cast to bf16, host-prepared
    xT_d = nc.dram_tensor("xT", [128, CT * N], BF16, kind="ExternalInput").ap()
    # [128, CT, 776]: wq(256 perm) | wk(256 perm) | wv(256) | mu_q(4) | mu_k(4)
    wq_d = nc.dram_tensor("wqkvT", [128, CT * 776], BF16, kind="ExternalInput").ap()
    wo_d = nc.dram_tensor("woT", [64, NHC * DIM], BF16, kind="ExternalInput").ap()
    # [128, NT, 256]: C2q | S2q | C2k | S2k  (gains, q-scale folded in)
    tab_d = nc.dram_tensor("tab", [128, NT * 256], BF16, kind="ExternalInput").ap()
    if with_qkv_bias:
        b_d = nc.dram_tensor("brow", [1, 776], BF16, kind="ExternalInput").ap()
    if with_ln_bias:
        tln_d = nc.dram_tensor("tln", [128, NT * 512], BF16, kind="ExternalInput").ap()
    out_d = nc.dram_tensor("outp", [N, DIM], F32, kind="ExternalOutput").ap()

    with tile.TileContext(nc) as tc, ExitStack() as ctx:
        consts = ctx.enter_context(tc.tile_pool(name="consts", bufs=1))
        pers = ctx.enter_context(tc.tile_pool(name="pers", bufs=1))
        stage = ctx.enter_context(tc.tile_pool(name="stage", bufs=6))
        small = ctx.enter_context(tc.tile_pool(name="small", bufs=6))
        exps = ctx.enter_context(tc.tile_pool(name="exps", bufs=12))
        ps = ctx.enter_context(tc.tile_pool(name="ps", bufs=6, space="PSUM"))
        psO = ctx.enter_context(tc.tile_pool(name="psO", bufs=2, space="PSUM"))

        # constants (xT/wq split per ct chunk so the first B1 matmuls can
        # start as soon as the first chunks land)
        xT_sb = consts.tile([128, CT, N], BF16)
        wq_sb = consts.tile([128, CT, 776], BF16)
        xT_r = xT_d.rearrange("p (t f) -> p t f", t=CT)
        wq_r = wq_d.rearrange("p (t f) -> p t f", t=CT)
        for ct in range(CT):
            nc.sync.dma_start(wq_sb[:, ct], wq_r[:, ct])
            nc.sync.dma_start(xT_sb[:, ct], xT_r[:, ct])
        wo_sb = consts.tile([64, NHC, DIM], BF16)
        nc.sync.dma_start(wo_sb[:], wo_d.rearrange("p (t f) -> p t f", t=NHC))
        tab_sb = consts.tile([128, NT, 256], BF16)
        nc.sync.dma_start(tab_sb[:], tab_d.rearrange("p (t f) -> p t f", t=NT))
        # row 64 is the K=1 lhsT for the reciprocal-replicate matmul fallback
        onesb_sb = consts.tile([65, 128], BF16)
        nc.vector.memset(onesb_sb[:], 1.0)
        if with_qkv_bias:
            b_sb = consts.tile([1, 776], BF16)
            nc.sync.dma_start(b_sb[:], b_d)
            ones_sb = consts.tile([1, 128], BF16)
            nc.vector.memset(ones_sb[:], 1.0)
        if with_ln_bias:
            tln_sb = consts.tile([128, NT, 512], BF16)
            nc.sync.dma_start(tln_sb[:], tln_d.rearrange("p (t f) -> p t f", t=NT))

        # persistent intermediates
        qT = [pers.tile([128, N], BF16, name=f"qT{i}") for i in range(2)]
        kT = [pers.tile([128, N], BF16, name=f"kT{i}") for i in range(2)]
        oT = [pers.tile([64, N], BF16, name=f"oTh{i}") for i in range(NHC)]
        # v with a ones column per head: PV row 64 is the softmax denominator
        v_sb = pers.tile([128, KT, NHC, 65], BF16)
        t_all = pers.tile([128, NT, 8, HD], BF16)
        ssq_all = pers.tile([128, NT, 8], F32)
        rs_sb = pers.tile([128, NT, 8], F32)
        rs2_sb = pers.tile([128, NT, 8], F32)
        dsq_all = pers.tile([128, NT, 8], F32, name="rsq_d")

        nc.vector.memset(v_sb[:, :, :, 64], 1.0)

        def b1(nt):
            qkv_ps = ps.tile([128, 512], F32, tag="big", name="qkv")
            qkv2_ps = ps.tile([128, 512], F32, tag="big", name="qkv2")
            for j0, j1, pstile in ((0, 512, qkv_ps), (512, 776, qkv2_ps)):
                w = j1 - j0
                for ct in range(CT):
                    nc.tensor.matmul(
                        pstile[:, 0:w],
                        lhsT=xT_sb[:, ct, nt * 128 : (nt + 1) * 128],
                        rhs=wq_sb[:, ct, j0:j1],
                        start=(ct == 0),
                        stop=(ct == CT - 1) and not with_qkv_bias,
                    )
                if with_qkv_bias:
                    nc.tensor.matmul(
                        pstile[:, 0:w],
                        lhsT=ones_sb[:],
                        rhs=b_sb[:, j0:j1],
                        start=False,
                        stop=True,
                    )
            mu = small.tile([128, 8], F32, name="mu")
            nc.vector.tensor_copy(mu[:], qkv2_ps[:, 256:264])
            nc.vector.tensor_tensor(
                t_all[:, nt],
                qkv_ps[:, 0:512].rearrange("p (h d) -> p h d", h=8),
                mu.unsqueeze(2).to_broadcast((128, 8, HD)),
                mybir.AluOpType.subtract,
            )
            # v evacuation on the (otherwise idle in phase B) scalar engine
            nc.scalar.copy(
                v_sb[:, nt, :, 0:64],
                qkv2_ps[:, 0:256].rearrange("p (h d) -> p h d", h=NHC),
            )
            # sum-of-squares on the (otherwise idle) GPSIMD engine to keep
            # the DVE free for the rope chain
            sq = stage.tile([128, 8, HD], BF16, name="sq")
            nc.gpsimd.tensor_mul(sq[:], t_all[:, nt], t_all[:, nt])
            nc.vector.tensor_reduce(
                ssq_all[:, nt], sq[:], axis=mybir.AxisListType.X, op=mybir.AluOpType.add
            )

        def rsqrt_batch(g):
            # rs = 1/sqrt(ssq/HD + eps) on DVE for nt in [4g, 4g+4)
            sl = slice(4 * g, 4 * g + 4)
            FLAT = 4 * 8
            d_t = dsq_all[:, sl].rearrange("p a b -> p (a b)")
            nc.vector.tensor_scalar(
                d_t, ssq_all[:, sl].rearrange("p a b -> p (a b)"), 1.0 / HD, EPS,
                mybir.AluOpType.mult, mybir.AluOpType.add,
            )
            fi = small.tile([128, FLAT], F32, name="rsq_fi")
            nc.vector.tensor_copy(fi[:], d_t.bitcast(I32))  # int32 -> f32 convert
            nc.vector.tensor_scalar(
                fi[:], fi[:], -0.5, RSQRT_MAGIC, mybir.AluOpType.mult, mybir.AluOpType.add
            )
            yi = small.tile([128, FLAT], I32, name="rsq_yi")
            nc.vector.tensor_copy(yi[:], fi[:])  # f32 -> int32 convert
            y = yi[:].bitcast(F32)
            h_t = small.tile([128, FLAT], F32, name="rsq_h")
            for _ in range(3):
                nc.vector.tensor_mul(h_t[:], y, y)
                nc.vector.tensor_mul(h_t[:], h_t[:], d_t)
                nc.vector.tensor_scalar(
                    h_t[:], h_t[:], -0.5, 1.5, mybir.AluOpType.mult, mybir.AluOpType.add
                )
                nc.vector.tensor_mul(y, y, h_t[:])
            nc.vector.tensor_copy(rs_sb[:, sl].rearrange("p a b -> p (a b)"), y)
            nc.vector.tensor_scalar_mul(
                rs2_sb[:, sl].rearrange("p a b -> p (a b)"), y, EXP_A
            )

        def b2(nt):
            t3 = t_all[:, nt]  # [p, 8, 64] bf16
            u = stage.tile([128, 8, HD], BF16, name="u")
            w = stage.tile([128, 8, HD], BF16, name="w")
            for side, tcol in ((0, 0), (1, 128)):
                hs = slice(side * 4, side * 4 + 4)
                nc.vector.tensor_mul(
                    u[:, hs, :],
                    t3[:, hs, :],
                    tab_sb[:, nt, tcol : tcol + 64].unsqueeze(1).to_broadcast((128, 4, HD)),
                )
                for half in (0, 1):
                    d_out = slice(half * 32, half * 32 + 32)
                    d_in = slice((1 - half) * 32, (1 - half) * 32 + 32)
                    nc.vector.tensor_mul(
                        w[:, hs, d_out],
                        t3[:, hs, d_in],
                        tab_sb[:, nt, tcol + 64 + half * 32 : tcol + 96 + half * 32]
                        .unsqueeze(1)
                        .to_broadcast((128, 4, 32)),
                    )
            qk2 = stage.tile([128, 8, HD], BF16, name="qk2")
            nc.vector.tensor_add(qk2[:], u[:], w[:])
            if with_ln_bias:
                nc.vector.tensor_add(
                    qk2[:], qk2[:],
                    tln_sb[:, nt, :].rearrange("p (h d) -> p h d", h=8),
                )
            nc.vector.tensor_mul(
                qk2[:, 0:4, :],
                qk2[:, 0:4, :],
                rs_sb[:, nt, 0:4].unsqueeze(2).to_broadcast((128, 4, HD)),
            )
            flat = qk2.rearrange("p h d -> p (h d)")
            for pair in range(2):
                # q transposes on the sync DGE, k transposes on the scalar DGE
                nc.sync.dma_start_transpose(
                    qT[pair][:, nt * 128 : (nt + 1) * 128],
                    flat[:, pair * 128 : (pair + 1) * 128],
                )
                nc.scalar.dma_start_transpose(
                    kT[pair][:, nt * 128 : (nt + 1) * 128],
                    flat[:, 256 + pair * 128 : 256 + (pair + 1) * 128],
                )

        # ---- phase B: 4-nt batches so B2/transposes pipeline behind B1 ----
        for g in range(NT // 4):
            for nt in range(4 * g, 4 * g + 4):
                b1(nt)
            rsqrt_batch(g)
            for nt in range(4 * g, 4 * g + 4):
                b2(nt)

        # ---- phase C: software-pipelined (qb, pair) blocks ----
        # per block: S+exp for all (kt, hh); PV of the previous block
        # interleaved at kt granularity; then the previous block's tail.
        blocks = [(qb, pair) for qb in range(QB) for pair in range(2)]

        def outproj(qb):
            for nt in range(qb * NTQ, (qb + 1) * NTQ):
                op = ps.tile([128, 512], F32, tag="big", name="op")
                for h in range(NHC):
                    nc.tensor.matmul(
                        op[:],
                        lhsT=oT[h][:, nt * 128 : (nt + 1) * 128],
                        rhs=wo_sb[:, h, :],
                        start=(h == 0),
                        stop=(h == NHC - 1),
                    )
                ot = stage.tile([128, DIM], F32, name="ot")
                nc.scalar.copy(ot[:], op[:])
                nc.sync.dma_start(out_d[nt * 128 : (nt + 1) * 128, :], ot[:])

        LAG = 3  # PV trails S by this many kt within a block

        def emit_tail(tqb, tpair, toT_ps):
            # normalize block (tqb, tpair)'s oT by the denominator row
            for hh in range(2):
                ph = 2 * tpair + hh
                # 1/den = exp(-ln(den)) on ACT: Ln and Exp live in the same
                # activation table set (natural_log_exp_and_others), so this
                # costs two small ACT ops and no table switch.
                lden = small.tile([65, QBW], F32, name="lden")
                nc.scalar.activation(
                    lden[64:65, :], toT_ps[hh][64:65, :],
                    mybir.ActivationFunctionType.Ln,
                )
                rec_row = small.tile([65, QBW], F32, name="rec_row")
                nc.scalar.activation(
                    rec_row[64:65, :], lden[64:65, :],
                    mybir.ActivationFunctionType.Exp, scale=-1.0,
                )
                # PE replicate of the (bf16-cast) reciprocal row
                rec16 = small.tile([65, QBW], BF16, name="rec16")
                nc.vector.tensor_copy(rec16[64:65, :], rec_row[64:65, :])
                rep_ps = ps.tile([128, 512], F32, tag="big", name="rep")
                nc.tensor.matmul(
                    rep_ps[0:64, :],
                    lhsT=onesb_sb[64:65, 0:64],
                    rhs=rec16[64:65, :],
                    start=True,
                    stop=True,
                )
                rec_bc = stage.tile([64, QBW], F32, name="rec_bc")
                nc.vector.tensor_copy(rec_bc[:], rep_ps[0:64, :])
                nc.vector.tensor_mul(
                    oT[ph][:, tqb * QBW : (tqb + 1) * QBW],
                    toT_ps[hh][0:64, :],
                    rec_bc[:],
                )
            if tpair == 1:
                outproj(tqb)

        pending = None
        for qb, pair in blocks:
            oT_ps = []
            etiles = {}

            def pv(kt, hh):
                if kt == 0 and hh == 0:
                    oT_ps.extend(
                        psO.tile([128, 512], F32, tag="oT", name=f"oT{i}")
                        for i in range(2)
                    )
                h = 2 * pair + hh
                nc.tensor.matmul(
                    oT_ps[hh][0:65, :],
                    lhsT=v_sb[:, kt, h, :],
                    rhs=etiles.pop((kt, hh))[:],
                    start=(kt == 0),
                    stop=(kt == KT - 1),
                )

            for kt in range(KT):
                for hh in range(2):
                    h = 2 * pair + hh
                    dsl = slice(hh * 64, hh * 64 + 64)
                    s_ps = ps.tile([128, 512], F32, tag="big", name="s")
                    nc.tensor.matmul(
                        s_ps[:],
                        lhsT=kT[pair][dsl, kt * 128 : (kt + 1) * 128],
                        rhs=qT[pair][dsl, qb * QBW : (qb + 1) * QBW],
                        start=True,
                        stop=True,
                    )
                    e_sb = exps.tile([128, QBW], BF16, tag="expS", name="expS")
                    if hh == 1 and kt < DVE_EXP_KT:
                        # Schraudolph exp: one DVE op, int16 out holding
                        # the bf16 bit pattern of ~exp(rs*s)
                        nc.vector.tensor_scalar(
                            e_sb[:].bitcast(I16),
                            s_ps[:],
                            rs2_sb[:, kt, 4 + h : 5 + h],
                            EXP_B,
                            mybir.AluOpType.mult,
                            mybir.AluOpType.add,
                        )
                    else:
                        nc.scalar.activation(
                            e_sb[:], s_ps[:], mybir.ActivationFunctionType.Exp,
                            scale=rs_sb[:, kt, 4 + h : 5 + h],
                        )
                    etiles[(kt, hh)] = e_sb
                if kt == 2 and pending is not None:
                    # previous block's tail, emitted here so its ACT/DVE
                    # chain hides behind this block's S matmuls
                    emit_tail(*pending)
                    pending = None
                if kt >= LAG:
                    for hh in range(2):
                        pv(kt - LAG, hh)
            for kt in range(KT - LAG, KT):
                for hh in range(2):
                    pv(kt, hh)
            pending = (qb, pair, oT_ps)
        emit_tail(*pending)

    return nc


# ---------------------------------------------------------------------------
# host-side input prep
# ---------------------------------------------------------------------------

def _prep_core_inputs(c, x, Wqkv_w, Wqkv_b, qn_g, qn_b, kn_g, kn_b, out_w):
    bf16 = ml_dtypes.bfloat16
    b, hg = c // 2, c % 2
    heads = np.arange(4 * hg, 4 * hg + 4)
    perm = np.concatenate([np.arange(0, HD, 2), np.arange(1, HD, 2)])

    Wq = Wqkv_w[0 * DIM : 1 * DIM].reshape(NH, HD, DIM)[heads][:, perm, :]
    Wk = Wqkv_w[1 * DIM : 2 * DIM].reshape(NH, HD, DIM)[heads][:, perm, :]
    Wv = Wqkv_w[2 * DIM : 3 * DIM].reshape(NH, HD, DIM)[heads]
    WT = np.concatenate(
        [
            Wq.reshape(256, DIM).T,
            Wk.reshape(256, DIM).T,
            Wv.reshape(256, DIM).T,
            (Wq.sum(axis=1) / HD).T,
            (Wk.sum(axis=1) / HD).T,
        ],
        axis=1,
    )  # [512, 776]
    wqkvT = np.ascontiguousarray(
        WT.reshape(CT, 128, 776).transpose(1, 0, 2).reshape(128, CT * 776)
    ).astype(bf16)

    # x transposed to [c, n] and tiled [128, CT, N]
    xTn = x[b].T  # [512, 2048]
    xT = np.ascontiguousarray(
        xTn.reshape(CT, 128, N).transpose(1, 0, 2).reshape(128, CT * N)
    ).astype(bf16)

    inv = 1.0 / (THETA ** (np.arange(0, HD, 2, dtype=np.float64) / HD))
    ang = np.arange(N, dtype=np.float64)[:, None] * inv[None, :]
    cos = np.cos(ang)
    sin = np.sin(ang)
    C2 = np.concatenate([cos, cos], axis=1)
    S2 = np.concatenate([-sin, sin], axis=1)
    SH = lambda v: np.concatenate([v[HD // 2 :], v[: HD // 2]])
    sc = HD ** -0.5
    g_q, g_k = qn_g[perm], kn_g[perm]
    C2q = C2 * g_q[None, :] * sc
    S2q = S2 * SH(g_q)[None, :] * sc
    C2k = C2 * g_k[None, :]
    S2k = S2 * SH(g_k)[None, :]
    tabN = np.concatenate([C2q, S2q, C2k, S2k], axis=1)  # [N, 256]
    tab = np.ascontiguousarray(
        tabN.reshape(NT, 128, 256).transpose(1, 0, 2).reshape(128, NT * 256)
    ).astype(bf16)

    # per-head Wo^T blocks [64, 512], stacked along free: [64, NHC*512]
    Wo = out_w.reshape(DIM, NH, HD)[:, heads, :]  # [512, 4, 64]
    woT = np.ascontiguousarray(
        Wo.transpose(1, 2, 0).reshape(NHC, HD, DIM).transpose(1, 0, 2).reshape(HD, NHC * DIM)
    ).astype(bf16)

    m = {"xT": xT, "wqkvT": wqkvT, "woT": woT, "tab": tab}

    if np.any(Wqkv_b != 0):
        bq = Wqkv_b[0 * DIM : 1 * DIM].reshape(NH, HD)[heads][:, perm]
        bk = Wqkv_b[1 * DIM : 2 * DIM].reshape(NH, HD)[heads][:, perm]
        bv = Wqkv_b[2 * DIM : 3 * DIM].reshape(NH, HD)[heads]
        brow = np.concatenate(
            [bq.ravel(), bk.ravel(), bv.ravel(), bq.mean(1), bk.mean(1)]
        )[None, :]
        m["brow"] = brow.astype(bf16)
    if np.any(qn_b != 0) or np.any(kn_b != 0):
        b_q, b_k = qn_b[perm], kn_b[perm]
        Tq = (C2 * b_q[None, :] + S2 * SH(b_q)[None, :]) * sc
        Tk = C2 * b_k[None, :] + S2 * SH(b_k)[None, :]
        tlnN = np.concatenate([np.tile(Tq, (1, 4)), np.tile(Tk, (1, 4))], axis=1)
        m["tln"] = np.ascontiguousarray(
            tlnN.reshape(NT, 128, 512).transpose(1, 0, 2).reshape(128, NT * 512)
        ).astype(bf16)
    return m


_PROGRAM_CACHE = {}


def _get_program(with_qkv_bias, with_ln_bias, legalize=True):
    key = (with_qkv_bias, with_ln_bias, legalize)
    if key not in _PROGRAM_CACHE:
        nc = build_program(with_qkv_bias, with_ln_bias)
        if legalize:
            legalize_sync_waits(nc, 1)
        _PROGRAM_CACHE[key] = nc
    return _PROGRAM_CACHE[key]


def _run(inputs, trace=False):
    x = np.asarray(inputs["x"], np.float32)
    Wqkv_w = np.asarray(inputs["Wqkv_w"], np.float32)
    Wqkv_b = np.asarray(inputs["Wqkv_b"], np.float32)
    qn_g = np.asarray(inputs["qn_g"], np.float32)
    qn_b = np.asarray(inputs["qn_b"], np.float32)
    kn_g = np.asarray(inputs["kn_g"], np.float32)
    kn_b = np.asarray(inputs["kn_b"], np.float32)
    out_w = np.asarray(inputs["out_w"], np.float32)
    out_b = np.asarray(inputs["out_b"], np.float32)

    import time as _time

    _t = _time.time()
    in_maps = [
        _prep_core_inputs(c, x, Wqkv_w, Wqkv_b, qn_g, qn_b, kn_g, kn_b, out_w)
        for c in range(8)
    ]
    print(f"[kernel] host prep {_time.time()-_t:.1f}s", flush=True)
    _t = _time.time()
    nc = _get_program("brow" in in_maps[0], "tln" in in_maps[0])
    print(f"[kernel] program {_time.time()-_t:.1f}s", flush=True)
    _t = _time.time()
    res = run_bass_kernel_spmd(nc, in_maps, list(range(8)), trace=trace)
    print(f"[kernel] run {_time.time()-_t:.1f}s", flush=True)

    B = x.shape[0]
    bv = Wqkv_b[2 * DIM : 3 * DIM]
    out_bias = out_b + out_w @ bv
    out = np.empty((B, N, DIM), np.float32)
    for b in range(B):
        out[b] = res.results[2 * b]["outp"] + res.results[2 * b + 1]["outp"] + out_bias
    return out, res


def kernel(**inputs):
    out, _ = _run(inputs, trace=False)
    return out


# revision 29
# speedup vs baseline: 1.1835x; 1.1835x over previous
"""Trainium2 Bass kernel for nn_Attention_11836929868370.

8-core sharding: core c -> batch b = c//2, head group hg = c%2 (4 of 8 heads).
Each core computes its 4 heads' attention and a partial output projection;
the host sums the two partials per batch and adds the output bias.

Per-core pipeline (all matmuls bf16, accumulation fp32 in PSUM):
  B (4-nt batches, pipelined): qkv = xT.T @ WqkvT with host-built "mean
      columns" so the per-head LN mean comes out of the matmul for free;
      DVE evacuates t=(q|k)-mu, ACT evacuates v; batched Newton rsqrt per
      4-nt group (so rope/transposes start early); RoPE via rotate-half with
      pre-permuted weights; q''/k'' transposed to [d, n] via xbar DMA
      transposes on the sync (q) and scalar (k) queues.
  C (software-pipelined blocks of (qb, head-pair)): per kt, S^T = k'' @ q''.T
      for both heads of the pair back-to-back — their lhsT base partitions
      (0/64) map to distinct PE row groups, so the K=64 matmuls run
      concurrently in the array. exp(S) runs on ACT for head hh=0 and as a
      one-instruction Schraudolph bitcast-exp on DVE (int16 out, bf16 bit
      pattern) for hh=1 — splitting the softmax-exp load across two engines.
      PV (stationary [v_h | ones]) for the PREVIOUS block interleaves with
      the current block's S matmuls to keep the PE dense; PV row 64 is the
      softmax denominator. Block tail: reciprocal_approx_fast on the
      denominator row, DMA-broadcast across 64 partitions, one DVE multiply.
  D.  out = oT.T @ WoT partial projection per q-block (ACT evacuation),
      DMA out. Host adds out_b (+ the v-bias contribution).
"""

import sys

if "/opt/trn_rl_repo" not in sys.path:
    sys.path.insert(0, "/opt/trn_rl_repo")

import math
from contextlib import ExitStack

import ml_dtypes
import numpy as np

import concourse.bass as bass
import concourse.mybir as mybir
import concourse.tile as tile
from concourse.bass_utils import run_bass_kernel_spmd

BF16 = mybir.dt.bfloat16
F32 = mybir.dt.float32
I16 = mybir.dt.int16
I32 = mybir.dt.int32

DIM, NH, HD = 512, 8, 64
N = 2048
EPS = 1e-6
THETA = 10000.0
NT = N // 128          # 16 n-tiles
CT = DIM // 128        # 4 c-tiles
NHC = 4                # heads per core
QB = 4                 # q blocks of 512
KT = NT                # key tiles
QBW = N // QB          # 512
NTQ = NT // QB         # 4 n-tiles per q block
RSQRT_MAGIC = float(0x5F3759DF)

# Schraudolph exp on DVE: e ~= bitcast_bf16(int16(s * rs * A + B))
EXP_A = 128.0 / math.log(2.0)
EXP_C = 5.0
EXP_B = 16256.0 - EXP_C
# number of kt tiles per block whose hh=1 exp runs on DVE (rest on ACT)
import os as _os

DVE_EXP_KT = int(_os.environ.get("K_DVE_EXP_KT", "16"))
USE_BOUNCE = int(_os.environ.get("K_USE_BOUNCE", "1"))


# ---------------------------------------------------------------------------
# sync-wait legalization: this walrus build rejects >1 sync wait per
# instruction; excess waits are hoisted onto NoOps placed immediately before
# the instruction on the same engine, which preserves ordering exactly.
# ---------------------------------------------------------------------------

def legalize_sync_waits(nc, max_waits=1):
    n = 0
    for fn in nc.m.functions:
        for bb in fn.blocks:
            new_insts = []
            for inst in bb.instructions:
                si = inst.sync_info
                if si is not None and si.on_wait and len(si.on_wait) > max_waits:
                    movable = [w for w in si.on_wait if w.wait_reg is None]
                    pinned = [w for w in si.on_wait if w.wait_reg is not None]
                    budget = max(max_waits - len(pinned), 0)
                    cut = len(movable) - budget
                    keep, excess = movable[cut:], movable[:cut]
                    for i in range(0, len(excess), max_waits):
                        nop = mybir.InstNoOp(
                            name=f"I-waitsplit-{n}",
                            engine=inst.engine,
                            text_hint="waitsplit",
                            sync_info=mybir.SyncInfo(
                                on_wait=excess[i : i + max_waits], on_update=[]
                            ),
                        )
                        n += 1
                        new_insts.append(nop)
                    si.on_wait = keep + pinned
                new_insts.append(inst)
            bb.instructions[:] = new_insts
    return n


# ---------------------------------------------------------------------------
# device program
# ---------------------------------------------------------------------------

def build_program(with_qkv_bias=False, with_ln_bias=False):
    nc = bass.Bass("TRN2", target_bir_lowering=False, debug=False, num_devices=8)

    # [128, CT, 2048]: x transposed (c on partitions) and cast to bf16, host-prepared
    xT_d = nc.dram_tensor("xT", [128, CT * N], BF16, kind="ExternalInput").ap()
    # [128, CT, 776]: wq(256 perm) | wk(256 perm) | wv(256) | mu_q(4) | mu_k(4)
    wq_d = nc.dram_tensor("wqkvT", [128, CT * 776], BF16, kind="ExternalInput").ap()
    wo_d = nc.dram_tensor("woT", [64, NHC * DIM], BF16, kind="ExternalInput").ap()
    # [128, NT, 256]: C2q | S2q | C2k | S2k  (gains, q-scale folded in)
    tab_d = nc.dram_tensor("tab", [128, NT * 256], BF16, kind="ExternalInput").ap()
    if with_qkv_bias:
        b_d = nc.dram_tensor("brow", [1, 776], BF16, kind="ExternalInput").ap()
    if with_ln_bias:
        tln_d = nc.dram_tensor("tln", [128, NT * 512], BF16, kind="ExternalInput").ap()
    out_d = nc.dram_tensor("outp", [N, DIM], F32, kind="ExternalOutput").ap()

    with tile.TileContext(nc) as tc, ExitStack() as ctx:
        consts = ctx.enter_context(tc.tile_pool(name="consts", bufs=1))
        pers = ctx.enter_context(tc.tile_pool(name="pers", bufs=1))
        stage = ctx.enter_context(tc.tile_pool(name="stage", bufs=6))
        small = ctx.enter_context(tc.tile_pool(name="small", bufs=6))
        exps = ctx.enter_context(tc.tile_pool(name="exps", bufs=12))
        ps = ctx.enter_context(tc.tile_pool(name="ps", bufs=6, space="PSUM"))
        psO = ctx.enter_context(tc.tile_pool(name="psO", bufs=2, space="PSUM"))

        # constants (xT/wq split per ct chunk so the first B1 matmuls can
        # start as soon as the first chunks land)
        xT_sb = consts.tile([128, CT, N], BF16)
        wq_sb = consts.tile([128, CT, 776], BF16)
        xT_r = xT_d.rearrange("p (t f) -> p t f", t=CT)
        wq_r = wq_d.rearrange("p (t f) -> p t f", t=CT)
        for ct in range(CT):
            nc.sync.dma_start(wq_sb[:, ct], wq_r[:, ct])
            nc.sync.dma_start(xT_sb[:, ct], xT_r[:, ct])
        wo_sb = consts.tile([64, NHC, DIM], BF16)
        nc.sync.dma_start(wo_sb[:], wo_d.rearrange("p (t f) -> p t f", t=NHC))
        tab_sb = consts.tile([128, NT, 256], BF16)
        nc.sync.dma_start(tab_sb[:], tab_d.rearrange("p (t f) -> p t f", t=NT))
        # row 64 is the K=1 lhsT for the reciprocal-replicate matmul fallback
        onesb_sb = consts.tile([65, 128], BF16)
        nc.vector.memset(onesb_sb[:], 1.0)
        if with_qkv_bias:
            b_sb = consts.tile([1, 776], BF16)
            nc.sync.dma_start(b_sb[:], b_d)
            ones_sb = consts.tile([1, 128], BF16)
            nc.vector.memset(ones_sb[:], 1.0)
        if with_ln_bias:
            tln_sb = consts.tile([128, NT, 512], BF16)
            nc.sync.dma_start(tln_sb[:], tln_d.rearrange("p (t f) -> p t f", t=NT))

        # persistent intermediates
        qT = [pers.tile([128, N], BF16, name=f"qT{i}") for i in range(2)]
        kT = [pers.tile([128, N], BF16, name=f"kT{i}") for i in range(2)]
        oT = [pers.tile([64, N], BF16, name=f"oTh{i}") for i in range(NHC)]
        # v with a ones column per head: PV row 64 is the softmax denominator
        v_sb = pers.tile([128, KT, NHC, 65], BF16)
        t_all = pers.tile([128, NT, 8, HD], BF16)
        ssq_all = pers.tile([128, NT, 8], F32)
        rs_sb = pers.tile([128, NT, 8], F32)
        rs2_sb = pers.tile([128, NT, 8], F32)
        dsq_all = pers.tile([128, NT, 8], F32, name="rsq_d")

        nc.vector.memset(v_sb[:, :, :, 64], 1.0)

        def b1(nt):
            qkv_ps = ps.tile([128, 512], F32, tag="big", name="qkv")
            qkv2_ps = ps.tile([128, 512], F32, tag="big", name="qkv2")
            for j0, j1, pstile in ((0, 512, qkv_ps), (512, 776, qkv2_ps)):
                w = j1 - j0
                for ct in range(CT):
                    nc.tensor.matmul(
                        pstile[:, 0:w],
                        lhsT=xT_sb[:, ct, nt * 128 : (nt + 1) * 128],
                        rhs=wq_sb[:, ct, j0:j1],
                        start=(ct == 0),
                        stop=(ct == CT - 1) and not with_qkv_bias,
                    )
                if with_qkv_bias:
                    nc.tensor.matmul(
                        pstile[:, 0:w],
                        lhsT=ones_sb[:],
                        rhs=b_sb[:, j0:j1],
                        start=False,
                        stop=True,
                    )
            mu = small.tile([128, 8], F32, name="mu")
            nc.vector.tensor_copy(mu[:], qkv2_ps[:, 256:264])
            nc.vector.tensor_tensor(
                t_all[:, nt],
                qkv_ps[:, 0:512].rearrange("p (h d) -> p h d", h=8),
                mu.unsqueeze(2).to_broadcast((128, 8, HD)),
                mybir.AluOpType.subtract,
            )
            # v evacuation on the (otherwise idle in phase B) scalar engine
            nc.scalar.copy(
                v_sb[:, nt, :, 0:64],
                qkv2_ps[:, 0:256].rearrange("p (h d) -> p h d", h=NHC),
            )
            sq = stage.tile([128, 8, HD], BF16, name="sq")
            nc.vector.tensor_mul(sq[:], t_all[:, nt], t_all[:, nt])
            nc.vector.tensor_reduce(
                ssq_all[:, nt], sq[:], axis=mybir.AxisListType.X, op=mybir.AluOpType.add
            )

        def rsqrt_batch(g):
            # rs = 1/sqrt(ssq/HD + eps) on DVE for nt in [4g, 4g+4)
            sl = slice(4 * g, 4 * g + 4)
            FLAT = 4 * 8
            d_t = dsq_all[:, sl].rearrange("p a b -> p (a b)")
            nc.vector.tensor_scalar(
                d_t, ssq_all[:, sl].rearrange("p a b -> p (a b)"), 1.0 / HD, EPS,
                mybir.AluOpType.mult, mybir.AluOpType.add,
            )
            fi = small.tile([128, FLAT], F32, name="rsq_fi")
            nc.vector.tensor_copy(fi[:], d_t.bitcast(I32))  # int32 -> f32 convert
            nc.vector.tensor_scalar(
                fi[:], fi[:], -0.5, RSQRT_MAGIC, mybir.AluOpType.mult, mybir.AluOpType.add
            )
            yi = small.tile([128, FLAT], I32, name="rsq_yi")
            nc.vector.tensor_copy(yi[:], fi[:])  # f32 -> int32 convert
            y = yi[:].bitcast(F32)
            h_t = small.tile([128, FLAT], F32, name="rsq_h")
            for _ in range(3):
                nc.vector.tensor_mul(h_t[:], y, y)
                nc.vector.tensor_mul(h_t[:], h_t[:], d_t)
                nc.vector.tensor_scalar(
                    h_t[:], h_t[:], -0.5, 1.5, mybir.AluOpType.mult, mybir.AluOpType.add
                )
                nc.vector.tensor_mul(y, y, h_t[:])
            nc.vector.tensor_copy(rs_sb[:, sl].rearrange("p a b -> p (a b)"), y)
            nc.vector.tensor_scalar_mul(
                rs2_sb[:, sl].rearrange("p a b -> p (a b)"), y, EXP_A
            )

        def b2(nt):
            t3 = t_all[:, nt]  # [p, 8, 64] bf16
            u = stage.tile([128, 8, HD], BF16, name="u")
            w = stage.tile([128, 8, HD], BF16, name="w")
            for side, tcol in ((0, 0), (1, 128)):
                hs = slice(side * 4, side * 4 + 4)
                nc.vector.tensor_mul(
                    u[:, hs, :],
                    t3[:, hs, :],
                    tab_sb[:, nt, tcol : tcol + 64].unsqueeze(1).to_broadcast((128, 4, HD)),
                )
                for half in (0, 1):
                    d_out = slice(half * 32, half * 32 + 32)
                    d_in = slice((1 - half) * 32, (1 - half) * 32 + 32)
                    nc.vector.tensor_mul(
                        w[:, hs, d_out],
                        t3[:, hs, d_in],
                        tab_sb[:, nt, tcol + 64 + half * 32 : tcol + 96 + half * 32]
                        .unsqueeze(1)
                        .to_broadcast((128, 4, 32)),
                    )
            qk2 = stage.tile([128, 8, HD], BF16, name="qk2")
            nc.vector.tensor_add(qk2[:], u[:], w[:])
            if with_ln_bias:
                nc.vector.tensor_add(
                    qk2[:], qk2[:],
                    tln_sb[:, nt, :].rearrange("p (h d) -> p h d", h=8),
                )
            nc.vector.tensor_mul(
                qk2[:, 0:4, :],
                qk2[:, 0:4, :],
                rs_sb[:, nt, 0:4].unsqueeze(2).to_broadcast((128, 4, HD)),
            )
            flat = qk2.rearrange("p h d -> p (h d)")
            for pair in range(2):
                # q transposes on the sync DGE, k transposes on the scalar DGE
                nc.sync.dma_start_transpose(
                    qT[pair][:, nt * 128 : (nt + 1) * 128],
                    flat[:, pair * 128 : (pair + 1) * 128],
                )
                nc.scalar.dma_start_transpose(
                    kT[pair][:, nt * 128 : (nt + 1) * 128],
                    flat[:, 256 + pair * 128 : 256 + (pair + 1) * 128],
                )

        # ---- phase B: 4-nt batches so B2/transposes pipeline behind B1 ----
        for g in range(NT // 4):
            for nt in range(4 * g, 4 * g + 4):
                b1(nt)
            rsqrt_batch(g)
            for nt in range(4 * g, 4 * g + 4):
                b2(nt)

        # ---- phase C: software-pipelined (qb, pair) blocks ----
        # per block: S+exp for all (kt, hh); PV of the previous block
        # interleaved at kt granularity; then the previous block's tail.
        blocks = [(qb, pair) for qb in range(QB) for pair in range(2)]

        def outproj(qb):
            for nt in range(qb * NTQ, (qb + 1) * NTQ):
                op = ps.tile([128, 512], F32, tag="big", name="op")
                for h in range(NHC):
                    nc.tensor.matmul(
                        op[:],
                        lhsT=oT[h][:, nt * 128 : (nt + 1) * 128],
                        rhs=wo_sb[:, h, :],
                        start=(h == 0),
                        stop=(h == NHC - 1),
                    )
                ot = stage.tile([128, DIM], F32, name="ot")
                nc.scalar.copy(ot[:], op[:])
                nc.sync.dma_start(out_d[nt * 128 : (nt + 1) * 128, :], ot[:])

        LAG = 3  # PV trails S by this many kt within a block

        def emit_tail_recip(tqb, tpair, toT_ps, recs):
            # 1/den = exp(-ln(den)) on ACT: Ln and Exp live in the same
            # activation table set (natural_log_exp_and_others), so this
            # costs two small ACT ops and no table switch. Cast to bf16 for
            # the PE replicate. Emitted at the head of the next block so
            # these ops run before that block's exps on the ACT/DVE queues.
            for hh in range(2):
                lden = small.tile([65, QBW], F32, name="lden")
                nc.scalar.activation(
                    lden[64:65, :], toT_ps[hh][64:65, :],
                    mybir.ActivationFunctionType.Ln,
                )
                rec_row = small.tile([65, QBW], F32, name="rec_row")
                nc.scalar.activation(
                    rec_row[64:65, :], lden[64:65, :],
                    mybir.ActivationFunctionType.Exp, scale=-1.0,
                )
                rec16 = small.tile([65, QBW], BF16, name="rec16")
                nc.vector.tensor_copy(rec16[64:65, :], rec_row[64:65, :])
                recs.append(rec16)

        def emit_tail_norm(tqb, tpair, toT_ps, recs):
            # PE replicate of the reciprocal row + normalize multiply
            for hh in range(2):
                ph = 2 * tpair + hh
                rep_ps = ps.tile([128, 512], F32, tag="big", name="rep")
                nc.tensor.matmul(
                    rep_ps[0:64, :],
                    lhsT=onesb_sb[64:65, 0:64],
                    rhs=recs[hh][64:65, :],
                    start=True,
                    stop=True,
                )
                rec_bc = stage.tile([64, QBW], F32, name="rec_bc")
                nc.vector.tensor_copy(rec_bc[:], rep_ps[0:64, :])
                nc.vector.tensor_mul(
                    oT[ph][:, tqb * QBW : (tqb + 1) * QBW],
                    toT_ps[hh][0:64, :],
                    rec_bc[:],
                )
            if tpair == 1:
                outproj(tqb)

        pending = None
        for qb, pair in blocks:
            oT_ps = []
            etiles = {}

            def pv(kt, hh):
                if kt == 0 and hh == 0:
                    oT_ps.extend(
                        psO.tile([128, 512], F32, tag="oT", name=f"oT{i}")
                        for i in range(2)
                    )
                h = 2 * pair + hh
                nc.tensor.matmul(
                    oT_ps[hh][0:65, :],
                    lhsT=v_sb[:, kt, h, :],
                    rhs=etiles.pop((kt, hh))[:],
                    start=(kt == 0),
                    stop=(kt == KT - 1),
                )

            for kt in range(KT):
                if kt == 0 and pending is not None:
                    emit_tail_recip(*pending)
                if kt == 2 and pending is not None:
                    emit_tail_norm(*pending)
                    pending = None
                for hh in range(2):
                    h = 2 * pair + hh
                    dsl = slice(hh * 64, hh * 64 + 64)
                    s_ps = ps.tile([128, 512], F32, tag="big", name="s")
                    nc.tensor.matmul(
                        s_ps[:],
                        lhsT=kT[pair][dsl, kt * 128 : (kt + 1) * 128],
                        rhs=qT[pair][dsl, qb * QBW : (qb + 1) * QBW],
                        start=True,
                        stop=True,
                    )
                    e_sb = exps.tile([128, QBW], BF16, tag="expS", name="expS")
                    if hh == 1 and kt < DVE_EXP_KT:
                        # Schraudolph exp: one DVE op, int16 out holding
                        # the bf16 bit pattern of ~exp(rs*s)
                        nc.vector.tensor_scalar(
                            e_sb[:].bitcast(I16),
                            s_ps[:],
                            rs2_sb[:, kt, 4 + h : 5 + h],
                            EXP_B,
                            mybir.AluOpType.mult,
                            mybir.AluOpType.add,
                        )
                    else:
                        nc.scalar.activation(
                            e_sb[:], s_ps[:], mybir.ActivationFunctionType.Exp,
                            scale=rs_sb[:, kt, 4 + h : 5 + h],
                        )
                    etiles[(kt, hh)] = e_sb
                if kt >= LAG:
                    for hh in range(2):
                        pv(kt - LAG, hh)
            for kt in range(KT - LAG, KT):
                for hh in range(2):
                    pv(kt, hh)
            pending = (qb, pair, oT_ps, [])
        emit_tail_recip(*pending)
        emit_tail_norm(*pending)

    return nc


# ---------------------------------------------------------------------------
# host-side input prep
# ---------------------------------------------------------------------------

def _prep_core_inputs(c, x, Wqkv_w, Wqkv_b, qn_g, qn_b, kn_g, kn_b, out_w):
    bf16 = ml_dtypes.bfloat16
    b, hg = c // 2, c % 2
    heads = np.arange(4 * hg, 4 * hg + 4)
    perm = np.concatenate([np.arange(0, HD, 2), np.arange(1, HD, 2)])

    Wq = Wqkv_w[0 * DIM : 1 * DIM].reshape(NH, HD, DIM)[heads][:, perm, :]
    Wk = Wqkv_w[1 * DIM : 2 * DIM].reshape(NH, HD, DIM)[heads][:, perm, :]
    Wv = Wqkv_w[2 * DIM : 3 * DIM].reshape(NH, HD, DIM)[heads]
    WT = np.concatenate(
        [
            Wq.reshape(256, DIM).T,
            Wk.reshape(256, DIM).T,
            Wv.reshape(256, DIM).T,
            (Wq.sum(axis=1) / HD).T,
            (Wk.sum(axis=1) / HD).T,
        ],
        axis=1,
    )  # [512, 776]
    wqkvT = np.ascontiguousarray(
        WT.reshape(CT, 128, 776).transpose(1, 0, 2).reshape(128, CT * 776)
    ).astype(bf16)

    # x transposed to [c, n] and tiled [128, CT, N]
    xTn = x[b].T  # [512, 2048]
    xT = np.ascontiguousarray(
        xTn.reshape(CT, 128, N).transpose(1, 0, 2).reshape(128, CT * N)
    ).astype(bf16)

    inv = 1.0 / (THETA ** (np.arange(0, HD, 2, dtype=np.float64) / HD))
    ang = np.arange(N, dtype=np.float64)[:, None] * inv[None, :]
    cos = np.cos(ang)
    sin = np.sin(ang)
    C2 = np.concatenate([cos, cos], axis=1)
    S2 = np.concatenate([-sin, sin], axis=1)
    SH = lambda v: np.concatenate([v[HD // 2 :], v[: HD // 2]])
    sc = HD ** -0.5
    g_q, g_k = qn_g[perm], kn_g[perm]
    C2q = C2 * g_q[None, :] * sc
    S2q = S2 * SH(g_q)[None, :] * sc
    C2k = C2 * g_k[None, :]
    S2k = S2 * SH(g_k)[None, :]
    tabN = np.concatenate([C2q, S2q, C2k, S2k], axis=1)  # [N, 256]
    tab = np.ascontiguousarray(
        tabN.reshape(NT, 128, 256).transpose(1, 0, 2).reshape(128, NT * 256)
    ).astype(bf16)

    # per-head Wo^T blocks [64, 512], stacked along free: [64, NHC*512]
    Wo = out_w.reshape(DIM, NH, HD)[:, heads, :]  # [512, 4, 64]
    woT = np.ascontiguousarray(
        Wo.transpose(1, 2, 0).reshape(NHC, HD, DIM).transpose(1, 0, 2).reshape(HD, NHC * DIM)
    ).astype(bf16)

    m = {"xT": xT, "wqkvT": wqkvT, "woT": woT, "tab": tab}

    if np.any(Wqkv_b != 0):
        bq = Wqkv_b[0 * DIM : 1 * DIM].reshape(NH, HD)[heads][:, perm]
        bk = Wqkv_b[1 * DIM : 2 * DIM].reshape(NH, HD)[heads][:, perm]
        bv = Wqkv_b[2 * DIM : 3 * DIM].reshape(NH, HD)[heads]
        brow = np.concatenate(
            [bq.ravel(), bk.ravel(), bv.ravel(), bq.mean(1), bk.mean(1)]
        )[None, :]
        m["brow"] = brow.astype(bf16)
    if np.any(qn_b != 0) or np.any(kn_b != 0):
        b_q, b_k = qn_b[perm], kn_b[perm]
        Tq = (C2 * b_q[None, :] + S2 * SH(b_q)[None, :]) * sc
        Tk = C2 * b_k[None, :] + S2 * SH(b_k)[None, :]
        tlnN = np.concatenate([np.tile(Tq, (1, 4)), np.tile(Tk, (1, 4))], axis=1)
        m["tln"] = np.ascontiguousarray(
            tlnN.reshape(NT, 128, 512).transpose(1, 0, 2).reshape(128, NT * 512)
        ).astype(bf16)
    return m


_PROGRAM_CACHE = {}


def _get_program(with_qkv_bias, with_ln_bias, legalize=True):
    key = (with_qkv_bias, with_ln_bias, legalize)
    if key not in _PROGRAM_CACHE:
        nc = build_program(with_qkv_bias, with_ln_bias)
        if legalize:
            legalize_sync_waits(nc, 1)
        _PROGRAM_CACHE[key] = nc
    return _PROGRAM_CACHE[key]


def _run(inputs, trace=False):
    x = np.asarray(inputs["x"], np.float32)
    Wqkv_w = np.asarray(inputs["Wqkv_w"], np.float32)
    Wqkv_b = np.asarray(inputs["Wqkv_b"], np.float32)
    qn_g = np.asarray(inputs["qn_g"], np.float32)
    qn_b = np.asarray(inputs["qn_b"], np.float32)
    kn_g = np.asarray(inputs["kn_g"], np.float32)
    kn_b = np.asarray(inputs["kn_b"], np.float32)
    out_w = np.asarray(inputs["out_w"], np.float32)
    out_b = np.asarray(inputs["out_b"], np.float32)

    import time as _time

    _t = _time.time()
    in_maps = [
        _prep_core_inputs(c, x, Wqkv_w, Wqkv_b, qn_g, qn_b, kn_g, kn_b, out_w)
        for c in range(8)
    ]
    print(f"[kernel] host prep {_time.time()-_t:.1f}s", flush=True)
    _t = _time.time()
    nc = _get_program("brow" in in_maps[0], "tln" in in_maps[0])
    print(f"[kernel] program {_time.time()-_t:.1f}s", flush=True)
    _t = _time.time()
    res = run_bass_kernel_spmd(nc, in_maps, list(range(8)), trace=trace)
    print(f"[kernel] run {_time.time()-_t:.1f}s", flush=True)

    B = x.shape[0]
    bv = Wqkv_b[2 * DIM : 3 * DIM]
    out_bias = out_b + out_w @ bv
    out = np.empty((B, N, DIM), np.float32)
    for b in range(B):
        out[b] = res.results[2 * b]["outp"] + res.results[2 * b + 1]["outp"] + out_bias
    return out, res


def kernel(**inputs):
    out, _ = _run(inputs, trace=False)
    return out


# revision 41
# speedup vs baseline: 1.3153x; 1.1113x over previous
"""Trainium2 Bass kernel for nn_Attention_11836929868370.

8-core sharding: core c -> batch b = c//2, head group hg = c%2 (4 of 8 heads).
Each core computes its 4 heads' attention and a partial output projection;
the host sums the two partials per batch and adds the output bias.

Per-core pipeline (all matmuls bf16, accumulation fp32 in PSUM):
  B (4-nt batches, pipelined): qkv = xT.T @ WqkvT with host-built "mean
      columns" so the per-head LN mean comes out of the matmul for free;
      DVE evacuates t=(q|k)-mu, ACT evacuates v; batched Newton rsqrt per
      4-nt group (so rope/transposes start early); RoPE via rotate-half with
      pre-permuted weights; q''/k'' transposed to [d, n] via xbar DMA
      transposes on the sync (q) and scalar (k) queues.
  C (software-pipelined blocks of (qb, head-pair)): per kt, S^T = k'' @ q''.T
      for both heads of the pair back-to-back — their lhsT base partitions
      (0/64) map to distinct PE row groups, so the K=64 matmuls run
      concurrently in the array. exp(S) runs on ACT for head hh=0 and as a
      one-instruction Schraudolph bitcast-exp on DVE (int16 out, bf16 bit
      pattern) for hh=1 — splitting the softmax-exp load across two engines.
      PV (stationary [v_h | ones]) for the PREVIOUS block interleaves with
      the current block's S matmuls to keep the PE dense; PV row 64 is the
      softmax denominator. Block tail: reciprocal_approx_fast on the
      denominator row, DMA-broadcast across 64 partitions, one DVE multiply.
  D.  out = oT.T @ WoT partial projection per q-block (ACT evacuation),
      DMA out. Host adds out_b (+ the v-bias contribution).
"""

import sys

if "/opt/trn_rl_repo" not in sys.path:
    sys.path.insert(0, "/opt/trn_rl_repo")

import math
from contextlib import ExitStack

import ml_dtypes
import numpy as np

import concourse.bass as bass
import concourse.mybir as mybir
import concourse.tile as tile
from concourse.bass_utils import run_bass_kernel_spmd

BF16 = mybir.dt.bfloat16
F32 = mybir.dt.float32
I16 = mybir.dt.int16
I32 = mybir.dt.int32

DIM, NH, HD = 512, 8, 64
N = 2048
EPS = 1e-6
THETA = 10000.0
NT = N // 128          # 16 n-tiles
CT = DIM // 128        # 4 c-tiles
NHC = 4                # heads per core
QB = 4                 # q blocks of 512
KT = NT                # key tiles
QBW = N // QB          # 512
NTQ = NT // QB         # 4 n-tiles per q block
RSQRT_MAGIC = float(0x5F3759DF)

# Schraudolph exp on DVE: e ~= bitcast_bf16(int16(s * rs * A + B))
EXP_A = 128.0 / math.log(2.0)
EXP_C = 5.0
EXP_B = 16256.0 - EXP_C
# number of kt tiles per block whose hh=1 exp runs on DVE (rest on ACT)
import os as _os

DVE_EXP_KT = int(_os.environ.get("K_DVE_EXP_KT", "16"))
USE_BOUNCE = int(_os.environ.get("K_USE_BOUNCE", "1"))


# ---------------------------------------------------------------------------
# sync-wait legalization: this walrus build rejects >1 sync wait per
# instruction; excess waits are hoisted onto NoOps placed immediately before
# the instruction on the same engine, which preserves ordering exactly.
# ---------------------------------------------------------------------------

def legalize_sync_waits(nc, max_waits=1):
    n = 0
    for fn in nc.m.functions:
        for bb in fn.blocks:
            new_insts = []
            for inst in bb.instructions:
                si = inst.sync_info
                if si is not None and si.on_wait and len(si.on_wait) > max_waits:
                    movable = [w for w in si.on_wait if w.wait_reg is None]
                    pinned = [w for w in si.on_wait if w.wait_reg is not None]
                    budget = max(max_waits - len(pinned), 0)
                    cut = len(movable) - budget
                    keep, excess = movable[cut:], movable[:cut]
                    for i in range(0, len(excess), max_waits):
                        nop = mybir.InstNoOp(
                            name=f"I-waitsplit-{n}",
                            engine=inst.engine,
                            text_hint="waitsplit",
                            sync_info=mybir.SyncInfo(
                                on_wait=excess[i : i + max_waits], on_update=[]
                            ),
                        )
                        n += 1
                        new_insts.append(nop)
                    si.on_wait = keep + pinned
                new_insts.append(inst)
            bb.instructions[:] = new_insts
    return n


# ---------------------------------------------------------------------------
# device program
# ---------------------------------------------------------------------------

def build_program(with_qkv_bias=False, with_ln_bias=False):
    nc = bass.Bass("TRN2", target_bir_lowering=False, debug=False, num_devices=8)

    # [128, CT, 2048]: x transposed (c on partitions) and cast to bf16, host-prepared
    xT_d = nc.dram_tensor("xT", [128, CT * N], BF16, kind="ExternalInput").ap()
    # [128, CT, 776]: wq(256 perm) | wk(256 perm) | wv(256) | mu_q(4) | mu_k(4)
    wq_d = nc.dram_tensor("wqkvT", [128, CT * 776], BF16, kind="ExternalInput").ap()
    # [128, 2, DIM]: head-pair-stacked Wo^T (rows 0:64 even head, 64:128 odd)
    wo_d = nc.dram_tensor("woT", [128, 2 * DIM], BF16, kind="ExternalInput").ap()
    # [128, NT, 256]: C2q | S2q | C2k | S2k  (gains, q-scale folded in)
    tab_d = nc.dram_tensor("tab", [128, NT * 256], BF16, kind="ExternalInput").ap()
    if with_qkv_bias:
        b_d = nc.dram_tensor("brow", [1, 776], BF16, kind="ExternalInput").ap()
    if with_ln_bias:
        tln_d = nc.dram_tensor("tln", [128, NT * 512], BF16, kind="ExternalInput").ap()
    out_d = nc.dram_tensor("outp", [N, DIM], F32, kind="ExternalOutput").ap()

    with tile.TileContext(nc) as tc, ExitStack() as ctx:
        consts = ctx.enter_context(tc.tile_pool(name="consts", bufs=1))
        pers = ctx.enter_context(tc.tile_pool(name="pers", bufs=1))
        stage = ctx.enter_context(tc.tile_pool(name="stage", bufs=6))
        small = ctx.enter_context(tc.tile_pool(name="small", bufs=6))
        exps = ctx.enter_context(tc.tile_pool(name="exps", bufs=12))
        ps = ctx.enter_context(tc.tile_pool(name="ps", bufs=6, space="PSUM"))
        psO = ctx.enter_context(tc.tile_pool(name="psO", bufs=2, space="PSUM"))

        # constants (xT/wq split per ct chunk so the first B1 matmuls can
        # start as soon as the first chunks land)
        xT_sb = consts.tile([128, CT, N], BF16)
        wq_sb = consts.tile([128, CT, 776], BF16)
        xT_r = xT_d.rearrange("p (t f) -> p t f", t=CT)
        wq_r = wq_d.rearrange("p (t f) -> p t f", t=CT)
        for ct in range(CT):
            nc.sync.dma_start(wq_sb[:, ct], wq_r[:, ct])
            nc.sync.dma_start(xT_sb[:, ct], xT_r[:, ct])
        wo_sb = consts.tile([128, 2, DIM], BF16)
        nc.sync.dma_start(wo_sb[:], wo_d.rearrange("p (t f) -> p t f", t=2))
        tab_sb = consts.tile([128, NT, 256], BF16)
        nc.sync.dma_start(tab_sb[:], tab_d.rearrange("p (t f) -> p t f", t=NT))
        # rows 63/64 are the K=1 lhsT for the reciprocal-replicate matmuls
        onesb_sb = consts.tile([128, 128], BF16)
        nc.vector.memset(onesb_sb[:], 1.0)
        if with_qkv_bias:
            b_sb = consts.tile([1, 776], BF16)
            nc.sync.dma_start(b_sb[:], b_d)
            ones_sb = consts.tile([1, 128], BF16)
            nc.vector.memset(ones_sb[:], 1.0)
        if with_ln_bias:
            tln_sb = consts.tile([128, NT, 512], BF16)
            nc.sync.dma_start(tln_sb[:], tln_d.rearrange("p (t f) -> p t f", t=NT))

        # persistent intermediates
        # qkT_all[:, 0:2] = q'' transposed (per pair), [:, 2:4] = k''
        qkT_all = pers.tile([128, 4, N], BF16, name="qkT")
        # oT for a head pair packed in one [128, N] tile: even head rows
        # 0:64, odd head rows 64:128 -> out-proj contracts K=128 in 2 MMs/nt
        oT_pair = [pers.tile([128, N], BF16, name=f"oTp{i}") for i in range(2)]
        # v per head with a ones column: even heads [v(64) | 1] -> PV rows
        # 0:64 = o, row 64 = den; odd heads have ones at col 32 and v at
        # 64:128 -> PV row 32 = den, rows 64:128 = o (den row 32 so the
        # replicate matmul's operands sit at a legal base partition)
        v_sb = pers.tile([128, KT, NHC, 128], BF16)
        t_all = pers.tile([128, NT, 8, HD], BF16)
        ssq_all = pers.tile([128, NT, 8], F32)
        rs_sb = pers.tile([128, NT, 8], F32)
        rs2_sb = pers.tile([128, NT, 8], F32)
        dsq_all = pers.tile([128, NT, 8], F32, name="rsq_d")

        nc.vector.memset(v_sb[:], 0.0)
        for h in range(NHC):
            nc.vector.memset(v_sb[:, :, h, 64 if h % 2 == 0 else 32], 1.0)

        def b1(nt):
            qkv_ps = ps.tile([128, 512], F32, tag="big", name="qkv")
            qkv2_ps = ps.tile([128, 512], F32, tag="big", name="qkv2")
            for j0, j1, pstile in ((0, 512, qkv_ps), (512, 776, qkv2_ps)):
                w = j1 - j0
                for ct in range(CT):
                    nc.tensor.matmul(
                        pstile[:, 0:w],
                        lhsT=xT_sb[:, ct, nt * 128 : (nt + 1) * 128],
                        rhs=wq_sb[:, ct, j0:j1],
                        start=(ct == 0),
                        stop=(ct == CT - 1) and not with_qkv_bias,
                    )
                if with_qkv_bias:
                    nc.tensor.matmul(
                        pstile[:, 0:w],
                        lhsT=ones_sb[:],
                        rhs=b_sb[:, j0:j1],
                        start=False,
                        stop=True,
                    )
            # mu + v evacuation on the (otherwise light in phase B) scalar
            # engine; the DVE keeps only the subtract and the ssq reduction
            mu = small.tile([128, 8], F32, name="mu")
            nc.scalar.copy(mu[:], qkv2_ps[:, 256:264])
            nc.vector.tensor_tensor(
                t_all[:, nt],
                qkv_ps[:, 0:512].rearrange("p (h d) -> p h d", h=8),
                mu.unsqueeze(2).to_broadcast((128, 8, HD)),
                mybir.AluOpType.subtract,
            )
            vsrc = qkv2_ps[:, 0:256].rearrange("p (h d) -> p h d", h=NHC)
            nc.scalar.copy(v_sb[:, nt, 0::2, 0:64], vsrc[:, 0::2])
            nc.scalar.copy(v_sb[:, nt, 1::2, 64:128], vsrc[:, 1::2])
            sq = stage.tile([128, 8, HD], BF16, name="sq")
            nc.vector.tensor_mul(sq[:], t_all[:, nt], t_all[:, nt])
            sqh = stage.tile([128, 8, HD // 2], BF16, name="sqh")
            nc.vector.tensor_add(sqh[:], sq[:, :, 0:32], sq[:, :, 32:64])
            nc.vector.tensor_reduce(
                ssq_all[:, nt], sqh[:], axis=mybir.AxisListType.X,
                op=mybir.AluOpType.add,
            )

        def rsqrt_batch(g):
            # rs = 1/sqrt(ssq/HD + eps) on DVE for nt in [4g, 4g+4)
            sl = slice(4 * g, 4 * g + 4)
            FLAT = 4 * 8
            d_t = dsq_all[:, sl].rearrange("p a b -> p (a b)")
            nc.vector.tensor_scalar(
                d_t, ssq_all[:, sl].rearrange("p a b -> p (a b)"), 1.0 / HD, EPS,
                mybir.AluOpType.mult, mybir.AluOpType.add,
            )
            fi = small.tile([128, FLAT], F32, name="rsq_fi")
            nc.vector.tensor_copy(fi[:], d_t.bitcast(I32))  # int32 -> f32 convert
            nc.vector.tensor_scalar(
                fi[:], fi[:], -0.5, RSQRT_MAGIC, mybir.AluOpType.mult, mybir.AluOpType.add
            )
            yi = small.tile([128, FLAT], I32, name="rsq_yi")
            nc.vector.tensor_copy(yi[:], fi[:])  # f32 -> int32 convert
            y = yi[:].bitcast(F32)
            h_t = small.tile([128, FLAT], F32, name="rsq_h")
            for _ in range(3):
                nc.vector.tensor_mul(h_t[:], y, y)
                nc.vector.tensor_mul(h_t[:], h_t[:], d_t)
                nc.vector.tensor_scalar(
                    h_t[:], h_t[:], -0.5, 1.5, mybir.AluOpType.mult, mybir.AluOpType.add
                )
                nc.vector.tensor_mul(y, y, h_t[:])
            nc.vector.tensor_copy(rs_sb[:, sl].rearrange("p a b -> p (a b)"), y)
            nc.vector.tensor_scalar_mul(
                rs2_sb[:, sl].rearrange("p a b -> p (a b)"), y, EXP_A
            )

        def b2(nt):
            t3 = t_all[:, nt]  # [p, 8, 64] bf16
            u = stage.tile([128, 8, HD], BF16, name="u")
            w = stage.tile([128, 8, HD], BF16, name="w")
            for side, tcol in ((0, 0), (1, 128)):
                hs = slice(side * 4, side * 4 + 4)
                nc.vector.tensor_mul(
                    u[:, hs, :],
                    t3[:, hs, :],
                    tab_sb[:, nt, tcol : tcol + 64].unsqueeze(1).to_broadcast((128, 4, HD)),
                )
                for half in (0, 1):
                    d_out = slice(half * 32, half * 32 + 32)
                    d_in = slice((1 - half) * 32, (1 - half) * 32 + 32)
                    nc.vector.tensor_mul(
                        w[:, hs, d_out],
                        t3[:, hs, d_in],
                        tab_sb[:, nt, tcol + 64 + half * 32 : tcol + 96 + half * 32]
                        .unsqueeze(1)
                        .to_broadcast((128, 4, 32)),
                    )
            qk2 = stage.tile([128, 8, HD], BF16, name="qk2")
            nc.vector.tensor_add(qk2[:], u[:], w[:])
            if with_ln_bias:
                nc.vector.tensor_add(
                    qk2[:], qk2[:],
                    tln_sb[:, nt, :].rearrange("p (h d) -> p h d", h=8),
                )
            nc.vector.tensor_mul(
                qk2[:, 0:4, :],
                qk2[:, 0:4, :],
                rs_sb[:, nt, 0:4].unsqueeze(2).to_broadcast((128, 4, HD)),
            )
            # one xbar call transposes all four 128-col chunks (q pair0/1,
            # k pair0/1) into qkT_all's tiled layout; alternate queues by nt
            flat = qk2.rearrange("p h d -> p (h d)")
            tq = nc.sync if nt % 2 == 0 else nc.scalar
            tq.dma_start_transpose(
                qkT_all[:, :, nt * 128 : (nt + 1) * 128], flat[:, 0:512]
            )

        # ---- phase B: 4-nt batches so B2/transposes pipeline behind B1 ----
        for g in range(NT // 4):
            for nt in range(4 * g, 4 * g + 4):
                b1(nt)
            rsqrt_batch(g)
            for nt in range(4 * g, 4 * g + 4):
                b2(nt)

        # ---- phase C: software-pipelined (qb, pair) blocks ----
        # per block: S+exp for all (kt, hh); PV of the previous block
        # interleaved at kt granularity; then the previous block's tail.
        blocks = [(qb, pair) for qb in range(QB) for pair in range(2)]

        def outproj(qb):
            for nt in range(qb * NTQ, (qb + 1) * NTQ):
                op = ps.tile([128, 512], F32, tag="big", name="op")
                for pr in range(2):
                    nc.tensor.matmul(
                        op[:],
                        lhsT=oT_pair[pr][:, nt * 128 : (nt + 1) * 128],
                        rhs=wo_sb[:, pr, :],
                        start=(pr == 0),
                        stop=(pr == 1),
                    )
                ot = stage.tile([128, DIM], F32, name="ot")
                nc.scalar.copy(ot[:], op[:])
                nc.sync.dma_start(out_d[nt * 128 : (nt + 1) * 128, :], ot[:])

        LAG = 3  # PV trails S by this many kt within a block

        def emit_tail_recip(tqb, tpair, toT_ps, recs):
            # 1/den = exp(-ln(den)) on ACT: Ln and Exp live in the same
            # activation table set (natural_log_exp_and_others), so this
            # costs two small ACT ops and no table switch. Cast to bf16 for
            # the PE replicate. Emitted at the head of the next block so
            # these ops run before that block's exps on the ACT/DVE queues.
            for hh in range(2):
                dr = 64 if hh == 0 else 32  # denominator row
                lden = small.tile([65, QBW], F32, name="lden")
                nc.scalar.activation(
                    lden[dr : dr + 1, :], toT_ps[hh][dr : dr + 1, :],
                    mybir.ActivationFunctionType.Ln,
                )
                rec_row = small.tile([65, QBW], F32, name="rec_row")
                nc.scalar.activation(
                    rec_row[dr : dr + 1, :], lden[dr : dr + 1, :],
                    mybir.ActivationFunctionType.Exp, scale=-1.0,
                )
                rec16 = small.tile([65, QBW], BF16, name="rec16")
                nc.vector.tensor_copy(rec16[dr : dr + 1, :], rec_row[dr : dr + 1, :])
                recs.append(rec16)

        def emit_tail_norm(tqb, tpair, toT_ps, recs):
            # PE replicate of the reciprocal row + normalize multiply into
            # the packed oT_pair tile (even head rows 0:64, odd rows 64:128)
            for hh in range(2):
                dr = 64 if hh == 0 else 32
                osl = slice(0, 64) if hh == 0 else slice(64, 128)
                rep_ps = ps.tile([128, 512], F32, tag="big", name="rep")
                nc.tensor.matmul(
                    rep_ps[osl, :],
                    lhsT=onesb_sb[dr : dr + 1, osl],
                    rhs=recs[hh][dr : dr + 1, :],
                    start=True,
                    stop=True,
                )
                rec_bc = stage.tile([128, QBW], F32, name="rec_bc")
                nc.vector.tensor_copy(rec_bc[osl, :], rep_ps[osl, :])
                nc.vector.tensor_mul(
                    oT_pair[tpair][osl, tqb * QBW : (tqb + 1) * QBW],
                    toT_ps[hh][osl, :],
                    rec_bc[osl, :],
                )
            if tpair == 1:
                outproj(tqb)

        pending = None
        for qb, pair in blocks:
            oT_ps = []
            etiles = {}

            def pv(kt, hh):
                if kt == 0 and hh == 0:
                    oT_ps.extend(
                        psO.tile([128, 512], F32, tag="oT", name=f"oT{i}")
                        for i in range(2)
                    )
                h = 2 * pair + hh
                # even head: [v|1] lhsT -> o rows 0:64, den row 64
                # odd head: ones@32|v@64:128 -> den row 32, o rows 64:128
                lhsT = v_sb[:, kt, h, 0:65] if hh == 0 else v_sb[:, kt, h, :]
                out = oT_ps[hh][0:65, :] if hh == 0 else oT_ps[hh][:, :]
                nc.tensor.matmul(
                    out,
                    lhsT=lhsT,
                    rhs=etiles.pop((kt, hh))[:],
                    start=(kt == 0),
                    stop=(kt == KT - 1),
                )

            for kt in range(KT):
                if kt == 0 and pending is not None:
                    emit_tail_recip(*pending)
                if kt == 2 and pending is not None:
                    emit_tail_norm(*pending)
                    pending = None
                for hh in range(2):
                    h = 2 * pair + hh
                    dsl = slice(hh * 64, hh * 64 + 64)
                    s_ps = ps.tile([128, 512], F32, tag="big", name="s")
                    nc.tensor.matmul(
                        s_ps[:],
                        lhsT=qkT_all[dsl, 2 + pair, kt * 128 : (kt + 1) * 128],
                        rhs=qkT_all[dsl, pair, qb * QBW : (qb + 1) * QBW],
                        start=True,
                        stop=True,
                    )
                    e_sb = exps.tile([128, QBW], BF16, tag="expS", name="expS")
                    if hh == 1 and kt < DVE_EXP_KT:
                        # Schraudolph exp: one DVE op, int16 out holding
                        # the bf16 bit pattern of ~exp(rs*s)
                        nc.vector.tensor_scalar(
                            e_sb[:].bitcast(I16),
                            s_ps[:],
                            rs2_sb[:, kt, 4 + h : 5 + h],
                            EXP_B,
                            mybir.AluOpType.mult,
                            mybir.AluOpType.add,
                        )
                    else:
                        nc.scalar.activation(
                            e_sb[:], s_ps[:], mybir.ActivationFunctionType.Exp,
                            scale=rs_sb[:, kt, 4 + h : 5 + h],
                        )
                    etiles[(kt, hh)] = e_sb
                if kt >= LAG:
                    for hh in range(2):
                        pv(kt - LAG, hh)
            for kt in range(KT - LAG, KT):
                for hh in range(2):
                    pv(kt, hh)
            pending = (qb, pair, oT_ps, [])
        emit_tail_recip(*pending)
        emit_tail_norm(*pending)

    return nc


# ---------------------------------------------------------------------------
# host-side input prep
# ---------------------------------------------------------------------------

def _prep_core_inputs(c, x, Wqkv_w, Wqkv_b, qn_g, qn_b, kn_g, kn_b, out_w):
    bf16 = ml_dtypes.bfloat16
    b, hg = c // 2, c % 2
    heads = np.arange(4 * hg, 4 * hg + 4)
    perm = np.concatenate([np.arange(0, HD, 2), np.arange(1, HD, 2)])

    Wq = Wqkv_w[0 * DIM : 1 * DIM].reshape(NH, HD, DIM)[heads][:, perm, :]
    Wk = Wqkv_w[1 * DIM : 2 * DIM].reshape(NH, HD, DIM)[heads][:, perm, :]
    Wv = Wqkv_w[2 * DIM : 3 * DIM].reshape(NH, HD, DIM)[heads]
    WT = np.concatenate(
        [
            Wq.reshape(256, DIM).T,
            Wk.reshape(256, DIM).T,
            Wv.reshape(256, DIM).T,
            (Wq.sum(axis=1) / HD).T,
            (Wk.sum(axis=1) / HD).T,
        ],
        axis=1,
    )  # [512, 776]
    wqkvT = np.ascontiguousarray(
        WT.reshape(CT, 128, 776).transpose(1, 0, 2).reshape(128, CT * 776)
    ).astype(bf16)

    # x transposed to [c, n] and tiled [128, CT, N]
    xTn = x[b].T  # [512, 2048]
    xT = np.ascontiguousarray(
        xTn.reshape(CT, 128, N).transpose(1, 0, 2).reshape(128, CT * N)
    ).astype(bf16)

    inv = 1.0 / (THETA ** (np.arange(0, HD, 2, dtype=np.float64) / HD))
    ang = np.arange(N, dtype=np.float64)[:, None] * inv[None, :]
    cos = np.cos(ang)
    sin = np.sin(ang)
    C2 = np.concatenate([cos, cos], axis=1)
    S2 = np.concatenate([-sin, sin], axis=1)
    SH = lambda v: np.concatenate([v[HD // 2 :], v[: HD // 2]])
    sc = HD ** -0.5
    g_q, g_k = qn_g[perm], kn_g[perm]
    C2q = C2 * g_q[None, :] * sc
    S2q = S2 * SH(g_q)[None, :] * sc
    C2k = C2 * g_k[None, :]
    S2k = S2 * SH(g_k)[None, :]
    tabN = np.concatenate([C2q, S2q, C2k, S2k], axis=1)  # [N, 256]
    tab = np.ascontiguousarray(
        tabN.reshape(NT, 128, 256).transpose(1, 0, 2).reshape(128, NT * 256)
    ).astype(bf16)

    # head-pair-stacked Wo^T: [128, 2, 512] -> rows 0:64 = even head of the
    # pair, rows 64:128 = odd head; pairs along the free dim
    Wo = out_w.reshape(DIM, NH, HD)[:, heads, :]  # [512, 4, 64]
    WoH = Wo.transpose(1, 2, 0)  # [4 heads, 64, 512]
    woT = np.ascontiguousarray(
        WoH.reshape(2, 2, HD, DIM)      # [pair, parity, 64, 512]
        .transpose(1, 2, 0, 3)          # [parity, 64, pair, 512]
        .reshape(128, 2 * DIM)
    ).astype(bf16)

    m = {"xT": xT, "wqkvT": wqkvT, "woT": woT, "tab": tab}

    if np.any(Wqkv_b != 0):
        bq = Wqkv_b[0 * DIM : 1 * DIM].reshape(NH, HD)[heads][:, perm]
        bk = Wqkv_b[1 * DIM : 2 * DIM].reshape(NH, HD)[heads][:, perm]
        bv = Wqkv_b[2 * DIM : 3 * DIM].reshape(NH, HD)[heads]
        brow = np.concatenate(
            [bq.ravel(), bk.ravel(), bv.ravel(), bq.mean(1), bk.mean(1)]
        )[None, :]
        m["brow"] = brow.astype(bf16)
    if np.any(qn_b != 0) or np.any(kn_b != 0):
        b_q, b_k = qn_b[perm], kn_b[perm]
        Tq = (C2 * b_q[None, :] + S2 * SH(b_q)[None, :]) * sc
        Tk = C2 * b_k[None, :] + S2 * SH(b_k)[None, :]
        tlnN = np.concatenate([np.tile(Tq, (1, 4)), np.tile(Tk, (1, 4))], axis=1)
        m["tln"] = np.ascontiguousarray(
            tlnN.reshape(NT, 128, 512).transpose(1, 0, 2).reshape(128, NT * 512)
        ).astype(bf16)
    return m


_PROGRAM_CACHE = {}


def _get_program(with_qkv_bias, with_ln_bias, legalize=True):
    key = (with_qkv_bias, with_ln_bias, legalize)
    if key not in _PROGRAM_CACHE:
        nc = build_program(with_qkv_bias, with_ln_bias)
        if legalize:
            legalize_sync_waits(nc, 1)
        _PROGRAM_CACHE[key] = nc
    return _PROGRAM_CACHE[key]


def _run(inputs, trace=False):
    x = np.asarray(inputs["x"], np.float32)
    Wqkv_w = np.asarray(inputs["Wqkv_w"], np.float32)
    Wqkv_b = np.asarray(inputs["Wqkv_b"], np.float32)
    qn_g = np.asarray(inputs["qn_g"], np.float32)
    qn_b = np.asarray(inputs["qn_b"], np.float32)
    kn_g = np.asarray(inputs["kn_g"], np.float32)
    kn_b = np.asarray(inputs["kn_b"], np.float32)
    out_w = np.asarray(inputs["out_w"], np.float32)
    out_b = np.asarray(inputs["out_b"], np.float32)

    import time as _time

    _t = _time.time()
    in_maps = [
        _prep_core_inputs(c, x, Wqkv_w, Wqkv_b, qn_g, qn_b, kn_g, kn_b, out_w)
        for c in range(8)
    ]
    print(f"[kernel] host prep {_time.time()-_t:.1f}s", flush=True)
    _t = _time.time()
    nc = _get_program("brow" in in_maps[0], "tln" in in_maps[0])
    print(f"[kernel] program {_time.time()-_t:.1f}s", flush=True)
    _t = _time.time()
    res = run_bass_kernel_spmd(nc, in_maps, list(range(8)), trace=trace)
    print(f"[kernel] run {_time.time()-_t:.1f}s", flush=True)

    B = x.shape[0]
    bv = Wqkv_b[2 * DIM : 3 * DIM]
    out_bias = out_b + out_w @ bv
    out = np.empty((B, N, DIM), np.float32)
    for b in range(B):
        out[b] = res.results[2 * b]["outp"] + res.results[2 * b + 1]["outp"] + out_bias
    return out, res


def kernel(**inputs):
    out, _ = _run(inputs, trace=False)
    return out


# revision 44
# speedup vs baseline: 1.3503x; 1.0266x over previous
"""Trainium2 Bass kernel for nn_Attention_11836929868370.

8-core sharding: core c -> batch b = c//2, head group hg = c%2 (4 of 8 heads).
Each core computes its 4 heads' attention and a partial output projection;
the host sums the two partials per batch and adds the output bias.

Per-core pipeline (all matmuls bf16, accumulation fp32 in PSUM):
  B (4-nt batches, pipelined): qkv = xT.T @ WqkvT with host-built "mean
      columns" so the per-head LN mean comes out of the matmul for free;
      DVE evacuates t=(q|k)-mu, ACT evacuates v; batched Newton rsqrt per
      4-nt group (so rope/transposes start early); RoPE via rotate-half with
      pre-permuted weights; q''/k'' transposed to [d, n] via xbar DMA
      transposes on the sync (q) and scalar (k) queues.
  C (software-pipelined blocks of (qb, head-pair)): per kt, S^T = k'' @ q''.T
      for both heads of the pair back-to-back — their lhsT base partitions
      (0/64) map to distinct PE row groups, so the K=64 matmuls run
      concurrently in the array. exp(S) runs on ACT for head hh=0 and as a
      one-instruction Schraudolph bitcast-exp on DVE (int16 out, bf16 bit
      pattern) for hh=1 — splitting the softmax-exp load across two engines.
      PV (stationary [v_h | ones]) for the PREVIOUS block interleaves with
      the current block's S matmuls to keep the PE dense; PV row 64 is the
      softmax denominator. Block tail: reciprocal_approx_fast on the
      denominator row, DMA-broadcast across 64 partitions, one DVE multiply.
  D.  out = oT.T @ WoT partial projection per q-block (ACT evacuation),
      DMA out. Host adds out_b (+ the v-bias contribution).
"""

import sys

if "/opt/trn_rl_repo" not in sys.path:
    sys.path.insert(0, "/opt/trn_rl_repo")

import math
from contextlib import ExitStack

import ml_dtypes
import numpy as np

import concourse.bass as bass
import concourse.mybir as mybir
import concourse.tile as tile
from concourse.bass_utils import run_bass_kernel_spmd

BF16 = mybir.dt.bfloat16
F32 = mybir.dt.float32
I16 = mybir.dt.int16
I32 = mybir.dt.int32

DIM, NH, HD = 512, 8, 64
N = 2048
EPS = 1e-6
THETA = 10000.0
NT = N // 128          # 16 n-tiles
CT = DIM // 128        # 4 c-tiles
NHC = 4                # heads per core
QB = 4                 # q blocks of 512
KT = NT                # key tiles
QBW = N // QB          # 512
NTQ = NT // QB         # 4 n-tiles per q block
RSQRT_MAGIC = float(0x5F3759DF)

# Schraudolph exp on DVE: e ~= bitcast_bf16(int16(s * rs * A + B))
EXP_A = 128.0 / math.log(2.0)
EXP_C = 5.0
EXP_B = 16256.0 - EXP_C
# number of kt tiles per block whose hh=1 exp runs on DVE (rest on ACT)
import os as _os

DVE_EXP_KT = int(_os.environ.get("K_DVE_EXP_KT", "13"))
USE_BOUNCE = int(_os.environ.get("K_USE_BOUNCE", "1"))


# ---------------------------------------------------------------------------
# sync-wait legalization: this walrus build rejects >1 sync wait per
# instruction; excess waits are hoisted onto NoOps placed immediately before
# the instruction on the same engine, which preserves ordering exactly.
# ---------------------------------------------------------------------------

def legalize_sync_waits(nc, max_waits=1):
    n = 0
    for fn in nc.m.functions:
        for bb in fn.blocks:
            new_insts = []
            for inst in bb.instructions:
                si = inst.sync_info
                if si is not None and si.on_wait and len(si.on_wait) > max_waits:
                    movable = [w for w in si.on_wait if w.wait_reg is None]
                    pinned = [w for w in si.on_wait if w.wait_reg is not None]
                    budget = max(max_waits - len(pinned), 0)
                    cut = len(movable) - budget
                    keep, excess = movable[cut:], movable[:cut]
                    for i in range(0, len(excess), max_waits):
                        nop = mybir.InstNoOp(
                            name=f"I-waitsplit-{n}",
                            engine=inst.engine,
                            text_hint="waitsplit",
                            sync_info=mybir.SyncInfo(
                                on_wait=excess[i : i + max_waits], on_update=[]
                            ),
                        )
                        n += 1
                        new_insts.append(nop)
                    si.on_wait = keep + pinned
                new_insts.append(inst)
            bb.instructions[:] = new_insts
    return n


# ---------------------------------------------------------------------------
# device program
# ---------------------------------------------------------------------------

def build_program(with_qkv_bias=False, with_ln_bias=False):
    nc = bass.Bass("TRN2", target_bir_lowering=False, debug=False, num_devices=8)

    # [128, CT, 2048]: x transposed (c on partitions) and cast to bf16, host-prepared
    xT_d = nc.dram_tensor("xT", [128, CT * N], BF16, kind="ExternalInput").ap()
    # [128, CT, 776]: wq(256 perm) | wk(256 perm) | wv(256) | mu_q(4) | mu_k(4)
    wq_d = nc.dram_tensor("wqkvT", [128, CT * 776], BF16, kind="ExternalInput").ap()
    # [128, 2, DIM]: head-pair-stacked Wo^T (rows 0:64 even head, 64:128 odd)
    wo_d = nc.dram_tensor("woT", [128, 2 * DIM], BF16, kind="ExternalInput").ap()
    # [128, NT, 256]: C2q | S2q | C2k | S2k  (gains, q-scale folded in)
    tab_d = nc.dram_tensor("tab", [128, NT * 256], BF16, kind="ExternalInput").ap()
    if with_qkv_bias:
        b_d = nc.dram_tensor("brow", [1, 776], BF16, kind="ExternalInput").ap()
    if with_ln_bias:
        tln_d = nc.dram_tensor("tln", [128, NT * 512], BF16, kind="ExternalInput").ap()
    out_d = nc.dram_tensor("outp", [N, DIM], F32, kind="ExternalOutput").ap()

    with tile.TileContext(nc) as tc, ExitStack() as ctx:
        consts = ctx.enter_context(tc.tile_pool(name="consts", bufs=1))
        pers = ctx.enter_context(tc.tile_pool(name="pers", bufs=1))
        stage = ctx.enter_context(tc.tile_pool(name="stage", bufs=6))
        small = ctx.enter_context(tc.tile_pool(name="small", bufs=6))
        exps = ctx.enter_context(tc.tile_pool(name="exps", bufs=12))
        ps = ctx.enter_context(tc.tile_pool(name="ps", bufs=6, space="PSUM"))
        psO = ctx.enter_context(tc.tile_pool(name="psO", bufs=2, space="PSUM"))

        # constants (xT/wq split per ct chunk so the first B1 matmuls can
        # start as soon as the first chunks land)
        xT_sb = consts.tile([128, CT, N], BF16)
        wq_sb = consts.tile([128, CT, 776], BF16)
        xT_r = xT_d.rearrange("p (t f) -> p t f", t=CT)
        wq_r = wq_d.rearrange("p (t f) -> p t f", t=CT)
        for ct in range(CT):
            nc.sync.dma_start(wq_sb[:, ct], wq_r[:, ct])
            nc.sync.dma_start(xT_sb[:, ct], xT_r[:, ct])
        wo_sb = consts.tile([128, 2, DIM], BF16)
        nc.sync.dma_start(wo_sb[:], wo_d.rearrange("p (t f) -> p t f", t=2))
        tab_sb = consts.tile([128, NT, 256], BF16)
        nc.sync.dma_start(tab_sb[:], tab_d.rearrange("p (t f) -> p t f", t=NT))
        # rows 63/64 are the K=1 lhsT for the reciprocal-replicate matmuls
        onesb_sb = consts.tile([128, 128], BF16)
        nc.vector.memset(onesb_sb[:], 1.0)
        if with_qkv_bias:
            b_sb = consts.tile([1, 776], BF16)
            nc.sync.dma_start(b_sb[:], b_d)
            ones_sb = consts.tile([1, 128], BF16)
            nc.vector.memset(ones_sb[:], 1.0)
        if with_ln_bias:
            tln_sb = consts.tile([128, NT, 512], BF16)
            nc.sync.dma_start(tln_sb[:], tln_d.rearrange("p (t f) -> p t f", t=NT))

        # persistent intermediates
        # qkT_all[:, 0:2] = q'' transposed (per pair), [:, 2:4] = k''
        qkT_all = pers.tile([128, 4, N], BF16, name="qkT")
        # oT for a head pair packed in one [128, N] tile: even head rows
        # 0:64, odd head rows 64:128 -> out-proj contracts K=128 in 2 MMs/nt
        oT_pair = [pers.tile([128, N], BF16, name=f"oTp{i}") for i in range(2)]
        # v per head with a ones column: even heads [v(64) | 1] -> PV rows
        # 0:64 = o, row 64 = den; odd heads have ones at col 32 and v at
        # 64:128 -> PV row 32 = den, rows 64:128 = o (den row 32 so the
        # replicate matmul's operands sit at a legal base partition)
        v_sb = pers.tile([128, KT, NHC, 128], BF16)
        t_all = pers.tile([128, NT, 8, HD], BF16)
        ssq_all = pers.tile([128, NT, 8], F32)
        rs_sb = pers.tile([128, NT, 8], F32)
        rs2_sb = pers.tile([128, NT, 8], F32)
        dsq_all = pers.tile([128, NT, 8], F32, name="rsq_d")

        nc.vector.memset(v_sb[:], 0.0)
        for h in range(NHC):
            nc.vector.memset(v_sb[:, :, h, 64 if h % 2 == 0 else 32], 1.0)

        def b1(nt):
            qkv_ps = ps.tile([128, 512], F32, tag="big", name="qkv")
            qkv2_ps = ps.tile([128, 512], F32, tag="big", name="qkv2")
            for j0, j1, pstile in ((0, 512, qkv_ps), (512, 776, qkv2_ps)):
                w = j1 - j0
                for ct in range(CT):
                    nc.tensor.matmul(
                        pstile[:, 0:w],
                        lhsT=xT_sb[:, ct, nt * 128 : (nt + 1) * 128],
                        rhs=wq_sb[:, ct, j0:j1],
                        start=(ct == 0),
                        stop=(ct == CT - 1) and not with_qkv_bias,
                    )
                if with_qkv_bias:
                    nc.tensor.matmul(
                        pstile[:, 0:w],
                        lhsT=ones_sb[:],
                        rhs=b_sb[:, j0:j1],
                        start=False,
                        stop=True,
                    )
            # mu + v evacuation on the (otherwise light in phase B) scalar
            # engine; the DVE keeps only the subtract and the ssq reduction
            mu = small.tile([128, 8], F32, name="mu")
            nc.scalar.copy(mu[:], qkv2_ps[:, 256:264])
            nc.vector.tensor_tensor(
                t_all[:, nt],
                qkv_ps[:, 0:512].rearrange("p (h d) -> p h d", h=8),
                mu.unsqueeze(2).to_broadcast((128, 8, HD)),
                mybir.AluOpType.subtract,
            )
            vsrc = qkv2_ps[:, 0:256].rearrange("p (h d) -> p h d", h=NHC)
            nc.scalar.copy(v_sb[:, nt, 0::2, 0:64], vsrc[:, 0::2])
            nc.scalar.copy(v_sb[:, nt, 1::2, 64:128], vsrc[:, 1::2])
            sq = stage.tile([128, 8, HD], BF16, name="sq")
            nc.vector.tensor_mul(sq[:], t_all[:, nt], t_all[:, nt])
            sqh = stage.tile([128, 8, HD // 2], BF16, name="sqh")
            nc.vector.tensor_add(sqh[:], sq[:, :, 0:32], sq[:, :, 32:64])
            nc.vector.tensor_reduce(
                ssq_all[:, nt], sqh[:], axis=mybir.AxisListType.X,
                op=mybir.AluOpType.add,
            )

        def rsqrt_batch(g):
            # rs = 1/sqrt(ssq/HD + eps) = exp(-0.5*ln(ssq/HD + eps)) on ACT
            # (Ln/Exp/Copy share one table set), keeping the DVE free
            sl = slice(4 * g, 4 * g + 4)
            FLAT = 4 * 8
            d_t = dsq_all[:, sl].rearrange("p a b -> p (a b)")
            nc.scalar.activation(
                d_t, ssq_all[:, sl].rearrange("p a b -> p (a b)"),
                mybir.ActivationFunctionType.Copy, bias=EPS, scale=1.0 / HD,
            )
            ld = small.tile([128, FLAT], F32, name="rsq_ld")
            nc.scalar.activation(ld[:], d_t, mybir.ActivationFunctionType.Ln)
            nc.scalar.activation(
                rs_sb[:, sl].rearrange("p a b -> p (a b)"), ld[:],
                mybir.ActivationFunctionType.Exp, scale=-0.5,
            )
            nc.scalar.mul(
                rs2_sb[:, sl].rearrange("p a b -> p (a b)"),
                rs_sb[:, sl].rearrange("p a b -> p (a b)"), EXP_A,
            )

        def b2(nt):
            t3 = t_all[:, nt]  # [p, 8, 64] bf16
            u = stage.tile([128, 8, HD], BF16, name="u")
            w = stage.tile([128, 8, HD], BF16, name="w")
            for side, tcol in ((0, 0), (1, 128)):
                hs = slice(side * 4, side * 4 + 4)
                nc.vector.tensor_mul(
                    u[:, hs, :],
                    t3[:, hs, :],
                    tab_sb[:, nt, tcol : tcol + 64].unsqueeze(1).to_broadcast((128, 4, HD)),
                )
                for half in (0, 1):
                    d_out = slice(half * 32, half * 32 + 32)
                    d_in = slice((1 - half) * 32, (1 - half) * 32 + 32)
                    nc.vector.tensor_mul(
                        w[:, hs, d_out],
                        t3[:, hs, d_in],
                        tab_sb[:, nt, tcol + 64 + half * 32 : tcol + 96 + half * 32]
                        .unsqueeze(1)
                        .to_broadcast((128, 4, 32)),
                    )
            qk2 = stage.tile([128, 8, HD], BF16, name="qk2")
            nc.vector.tensor_add(qk2[:], u[:], w[:])
            if with_ln_bias:
                nc.vector.tensor_add(
                    qk2[:], qk2[:],
                    tln_sb[:, nt, :].rearrange("p (h d) -> p h d", h=8),
                )
            nc.vector.tensor_mul(
                qk2[:, 0:4, :],
                qk2[:, 0:4, :],
                rs_sb[:, nt, 0:4].unsqueeze(2).to_broadcast((128, 4, HD)),
            )
            # one xbar call transposes all four 128-col chunks (q pair0/1,
            # k pair0/1) into qkT_all's tiled layout; alternate queues by nt
            flat = qk2.rearrange("p h d -> p (h d)")
            tq = nc.sync if nt % 2 == 0 else nc.scalar
            tq.dma_start_transpose(
                qkT_all[:, :, nt * 128 : (nt + 1) * 128], flat[:, 0:512]
            )

        # ---- phase B: 4-nt batches so B2/transposes pipeline behind B1 ----
        for g in range(NT // 4):
            for nt in range(4 * g, 4 * g + 4):
                b1(nt)
            rsqrt_batch(g)
            for nt in range(4 * g, 4 * g + 4):
                b2(nt)

        # ---- phase C: software-pipelined (qb, pair) blocks ----
        # per block: S+exp for all (kt, hh); PV of the previous block
        # interleaved at kt granularity; then the previous block's tail.
        blocks = [(qb, pair) for qb in range(QB) for pair in range(2)]

        def outproj(qb):
            for nt in range(qb * NTQ, (qb + 1) * NTQ):
                op = ps.tile([128, 512], F32, tag="big", name="op")
                for pr in range(2):
                    nc.tensor.matmul(
                        op[:],
                        lhsT=oT_pair[pr][:, nt * 128 : (nt + 1) * 128],
                        rhs=wo_sb[:, pr, :],
                        start=(pr == 0),
                        stop=(pr == 1),
                    )
                ot = stage.tile([128, DIM], F32, name="ot")
                nc.scalar.copy(ot[:], op[:])
                nc.sync.dma_start(out_d[nt * 128 : (nt + 1) * 128, :], ot[:])

        LAG = 3  # PV trails S by this many kt within a block

        def emit_tail_recip(tqb, tpair, toT_ps, recs):
            # 1/den = exp(-ln(den)) on ACT: Ln and Exp live in the same
            # activation table set (natural_log_exp_and_others), so this
            # costs two small ACT ops and no table switch. Cast to bf16 for
            # the PE replicate. Emitted at the head of the next block so
            # these ops run before that block's exps on the ACT/DVE queues.
            for hh in range(2):
                dr = 64 if hh == 0 else 32  # denominator row
                lden = small.tile([65, QBW], F32, name="lden")
                nc.scalar.activation(
                    lden[dr : dr + 1, :], toT_ps[hh][dr : dr + 1, :],
                    mybir.ActivationFunctionType.Ln,
                )
                rec16 = small.tile([65, QBW], BF16, name="rec16")
                nc.scalar.activation(
                    rec16[dr : dr + 1, :], lden[dr : dr + 1, :],
                    mybir.ActivationFunctionType.Exp, scale=-1.0,
                )
                recs.append(rec16)

        def emit_tail_norm(tqb, tpair, toT_ps, recs):
            # PE replicate of the reciprocal row + normalize multiply into
            # the packed oT_pair tile (even head rows 0:64, odd rows 64:128)
            for hh in range(2):
                dr = 64 if hh == 0 else 32
                osl = slice(0, 64) if hh == 0 else slice(64, 128)
                rep_ps = ps.tile([128, 512], F32, tag="big", name="rep")
                nc.tensor.matmul(
                    rep_ps[osl, :],
                    lhsT=onesb_sb[dr : dr + 1, osl],
                    rhs=recs[hh][dr : dr + 1, :],
                    start=True,
                    stop=True,
                )
                rec_bc = stage.tile([128, QBW], F32, name="rec_bc")
                nc.vector.tensor_copy(rec_bc[osl, :], rep_ps[osl, :])
                nc.vector.tensor_mul(
                    oT_pair[tpair][osl, tqb * QBW : (tqb + 1) * QBW],
                    toT_ps[hh][osl, :],
                    rec_bc[osl, :],
                )
            if tpair == 1:
                outproj(tqb)

        pending = None
        for qb, pair in blocks:
            oT_ps = []
            etiles = {}

            def pv(kt, hh):
                if kt == 0 and hh == 0:
                    oT_ps.extend(
                        psO.tile([128, 512], F32, tag="oT", name=f"oT{i}")
                        for i in range(2)
                    )
                h = 2 * pair + hh
                # even head: [v|1] lhsT -> o rows 0:64, den row 64
                # odd head: ones@32|v@64:128 -> den row 32, o rows 64:128
                lhsT = v_sb[:, kt, h, 0:65] if hh == 0 else v_sb[:, kt, h, :]
                out = oT_ps[hh][0:65, :] if hh == 0 else oT_ps[hh][:, :]
                nc.tensor.matmul(
                    out,
                    lhsT=lhsT,
                    rhs=etiles.pop((kt, hh))[:],
                    start=(kt == 0),
                    stop=(kt == KT - 1),
                )

            for kt in range(KT):
                if kt == 0 and pending is not None:
                    emit_tail_recip(*pending)
                if kt == 2 and pending is not None:
                    emit_tail_norm(*pending)
                    pending = None
                for hh in range(2):
                    h = 2 * pair + hh
                    dsl = slice(hh * 64, hh * 64 + 64)
                    s_ps = ps.tile([128, 512], F32, tag="big", name="s")
                    nc.tensor.matmul(
                        s_ps[:],
                        lhsT=qkT_all[dsl, 2 + pair, kt * 128 : (kt + 1) * 128],
                        rhs=qkT_all[dsl, pair, qb * QBW : (qb + 1) * QBW],
                        start=True,
                        stop=True,
                    )
                    e_sb = exps.tile([128, QBW], BF16, tag="expS", name="expS")
                    if hh == 1 and kt < DVE_EXP_KT:
                        # Schraudolph exp: one DVE op, int16 out holding
                        # the bf16 bit pattern of ~exp(rs*s)
                        nc.vector.tensor_scalar(
                            e_sb[:].bitcast(I16),
                            s_ps[:],
                            rs2_sb[:, kt, 4 + h : 5 + h],
                            EXP_B,
                            mybir.AluOpType.mult,
                            mybir.AluOpType.add,
                        )
                    else:
                        nc.scalar.activation(
                            e_sb[:], s_ps[:], mybir.ActivationFunctionType.Exp,
                            scale=rs_sb[:, kt, 4 + h : 5 + h],
                        )
                    etiles[(kt, hh)] = e_sb
                if kt >= LAG:
                    for hh in range(2):
                        pv(kt - LAG, hh)
            for kt in range(KT - LAG, KT):
                for hh in range(2):
                    pv(kt, hh)
            pending = (qb, pair, oT_ps, [])
        emit_tail_recip(*pending)
        emit_tail_norm(*pending)

    return nc


# ---------------------------------------------------------------------------
# host-side input prep
# ---------------------------------------------------------------------------

def _prep_core_inputs(c, x, Wqkv_w, Wqkv_b, qn_g, qn_b, kn_g, kn_b, out_w):
    bf16 = ml_dtypes.bfloat16
    b, hg = c // 2, c % 2
    heads = np.arange(4 * hg, 4 * hg + 4)
    perm = np.concatenate([np.arange(0, HD, 2), np.arange(1, HD, 2)])

    Wq = Wqkv_w[0 * DIM : 1 * DIM].reshape(NH, HD, DIM)[heads][:, perm, :]
    Wk = Wqkv_w[1 * DIM : 2 * DIM].reshape(NH, HD, DIM)[heads][:, perm, :]
    Wv = Wqkv_w[2 * DIM : 3 * DIM].reshape(NH, HD, DIM)[heads]
    WT = np.concatenate(
        [
            Wq.reshape(256, DIM).T,
            Wk.reshape(256, DIM).T,
            Wv.reshape(256, DIM).T,
            (Wq.sum(axis=1) / HD).T,
            (Wk.sum(axis=1) / HD).T,
        ],
        axis=1,
    )  # [512, 776]
    wqkvT = np.ascontiguousarray(
        WT.reshape(CT, 128, 776).transpose(1, 0, 2).reshape(128, CT * 776)
    ).astype(bf16)

    # x transposed to [c, n] and tiled [128, CT, N]
    xTn = x[b].T  # [512, 2048]
    xT = np.ascontiguousarray(
        xTn.reshape(CT, 128, N).transpose(1, 0, 2).reshape(128, CT * N)
    ).astype(bf16)

    inv = 1.0 / (THETA ** (np.arange(0, HD, 2, dtype=np.float64) / HD))
    ang = np.arange(N, dtype=np.float64)[:, None] * inv[None, :]
    cos = np.cos(ang)
    sin = np.sin(ang)
    C2 = np.concatenate([cos, cos], axis=1)
    S2 = np.concatenate([-sin, sin], axis=1)
    SH = lambda v: np.concatenate([v[HD // 2 :], v[: HD // 2]])
    sc = HD ** -0.5
    g_q, g_k = qn_g[perm], kn_g[perm]
    C2q = C2 * g_q[None, :] * sc
    S2q = S2 * SH(g_q)[None, :] * sc
    C2k = C2 * g_k[None, :]
    S2k = S2 * SH(g_k)[None, :]
    tabN = np.concatenate([C2q, S2q, C2k, S2k], axis=1)  # [N, 256]
    tab = np.ascontiguousarray(
        tabN.reshape(NT, 128, 256).transpose(1, 0, 2).reshape(128, NT * 256)
    ).astype(bf16)

    # head-pair-stacked Wo^T: [128, 2, 512] -> rows 0:64 = even head of the
    # pair, rows 64:128 = odd head; pairs along the free dim
    Wo = out_w.reshape(DIM, NH, HD)[:, heads, :]  # [512, 4, 64]
    WoH = Wo.transpose(1, 2, 0)  # [4 heads, 64, 512]
    woT = np.ascontiguousarray(
        WoH.reshape(2, 2, HD, DIM)      # [pair, parity, 64, 512]
        .transpose(1, 2, 0, 3)          # [parity, 64, pair, 512]
        .reshape(128, 2 * DIM)
    ).astype(bf16)

    m = {"xT": xT, "wqkvT": wqkvT, "woT": woT, "tab": tab}

    if np.any(Wqkv_b != 0):
        bq = Wqkv_b[0 * DIM : 1 * DIM].reshape(NH, HD)[heads][:, perm]
        bk = Wqkv_b[1 * DIM : 2 * DIM].reshape(NH, HD)[heads][:, perm]
        bv = Wqkv_b[2 * DIM : 3 * DIM].reshape(NH, HD)[heads]
        brow = np.concatenate(
            [bq.ravel(), bk.ravel(), bv.ravel(), bq.mean(1), bk.mean(1)]
        )[None, :]
        m["brow"] = brow.astype(bf16)
    if np.any(qn_b != 0) or np.any(kn_b != 0):
        b_q, b_k = qn_b[perm], kn_b[perm]
        Tq = (C2 * b_q[None, :] + S2 * SH(b_q)[None, :]) * sc
        Tk = C2 * b_k[None, :] + S2 * SH(b_k)[None, :]
        tlnN = np.concatenate([np.tile(Tq, (1, 4)), np.tile(Tk, (1, 4))], axis=1)
        m["tln"] = np.ascontiguousarray(
            tlnN.reshape(NT, 128, 512).transpose(1, 0, 2).reshape(128, NT * 512)
        ).astype(bf16)
    return m


_PROGRAM_CACHE = {}


def _get_program(with_qkv_bias, with_ln_bias, legalize=True):
    key = (with_qkv_bias, with_ln_bias, legalize)
    if key not in _PROGRAM_CACHE:
        nc = build_program(with_qkv_bias, with_ln_bias)
        if legalize:
            legalize_sync_waits(nc, 1)
        _PROGRAM_CACHE[key] = nc
    return _PROGRAM_CACHE[key]


def _run(inputs, trace=False):
    x = np.asarray(inputs["x"], np.float32)
    Wqkv_w = np.asarray(inputs["Wqkv_w"], np.float32)
    Wqkv_b = np.asarray(inputs["Wqkv_b"], np.float32)
    qn_g = np.asarray(inputs["qn_g"], np.float32)
    qn_b = np.asarray(inputs["qn_b"], np.float32)
    kn_g = np.asarray(inputs["kn_g"], np.float32)
    kn_b = np.asarray(inputs["kn_b"], np.float32)
    out_w = np.asarray(inputs["out_w"], np.float32)
    out_b = np.asarray(inputs["out_b"], np.float32)

    import time as _time

    _t = _time.time()
    in_maps = [
        _prep_core_inputs(c, x, Wqkv_w, Wqkv_b, qn_g, qn_b, kn_g, kn_b, out_w)
        for c in range(8)
    ]
    print(f"[kernel] host prep {_time.time()-_t:.1f}s", flush=True)
    _t = _time.time()
    nc = _get_program("brow" in in_maps[0], "tln" in in_maps[0])
    print(f"[kernel] program {_time.time()-_t:.1f}s", flush=True)
    _t = _time.time()
    res = run_bass_kernel_spmd(nc, in_maps, list(range(8)), trace=trace)
    print(f"[kernel] run {_time.time()-_t:.1f}s", flush=True)

    B = x.shape[0]
    bv = Wqkv_b[2 * DIM : 3 * DIM]
    out_bias = out_b + out_w @ bv
    out = np.empty((B, N, DIM), np.float32)
    for b in range(B):
        out[b] = res.results[2 * b]["outp"] + res.results[2 * b + 1]["outp"] + out_bias
    return out, res


def kernel(**inputs):
    out, _ = _run(inputs, trace=False)
    return out
